# revision 46
# baseline (speedup 1.0000x reference)
"""Trainium2 Bass kernel for CAConv2 (coordinate-attention + 3x3 conv block).

Shapes (hardcoded): x (8, 128, 128, 128) f32; data-parallel over batch,
one image per NeuronCore (8 cores).

Scheduling facts learned from HW traces / the cost model:
- PE streams conv matmuls at ~216ns/512 cols (~2.3 GHz effective) once
  warm; the 3x3 conv (32 blocks x 9 matmuls) is a ~63us floor and
  everything else must hide around it.
- Engines run their queues in order with a ~4-deep stalled-instruction
  bypass window: emission position is priority, dependencies are
  eligibility. A stalled matmul whose LDWEIGHTS has issued blocks ALL
  later matmuls, so PE matmuls with late feeders must be emitted at a
  queue position where their inputs are certainly ready (the B-path
  a_h matmuls are spliced between conv blocks 11 and 12).
- Pool (gpsimd) elementwise ops halve concurrent PE/DVE throughput
  (SBUF contention) and cannot touch PSUM; scalar-ptr / max-min Pool
  ops cost ~2us. Pool is left idle on purpose.
- DMA engines (16) sustain ~21 GB/s each (~300-340 GB/s aggregate); x
  (4MB bf16) needs ~13.5us of drain. All loads ride the sync HW ring
  (the gpsimd SW ring has ~1us wakeup and its completion semaphores
  stall behind trigger issuance); wct queues behind all of x on the
  same in-order ring. DMA triggers cost ~0.6-0.8us each on the issuing
  queue and completion semaphores fire ~1-2us after last data (engine
  stagger).
- ACT tables load per *set*: sigmoid set first for the critical a_w /
  a_h(0-63) activations; the single switch to the silu set lands in the
  ACT-idle window before the first conv Silu; the mid-conv a_h(64-127)
  sigmoid is computed as 0.5+0.5*tanh(z/2) (Tanh is in the silu set).
- Cross-engine dependency hops cost ~0.3-1us; run-to-run variance is
  ~+-1.5us (DMA ring behavior).
"""

import numpy as np
import ml_dtypes

import concourse.bacc as bacc
import concourse.tile as tile
from concourse import mybir
from concourse.bass import ds
from concourse.bass_utils import run_bass_kernel_spmd

BF16 = mybir.dt.bfloat16
F32 = mybir.dt.float32
C, H, W, MIP = 128, 128, 128, 8
WP = W + 4  # padded width: cols [2, 130) hold data, 0/1 and 130/131 are zero
HP = H + 2  # padded height: rows [1, 129) hold data
EPS = 1e-5
AF = mybir.ActivationFunctionType
ALU = mybir.AluOpType

_CACHE = {}


def build_nc():
    nc = bacc.Bacc(num_swdge_queues=1)
    xp = nc.declare_dram_parameter("x", [C, H * W], BF16, isOutput=False)
    w1ts = nc.declare_dram_parameter("w1ts", [C, 3 * MIP], BF16, isOutput=False)
    # wht | wwt packed side by side (one DMA trigger)
    whw = nc.declare_dram_parameter("whw", [MIP, 2 * C], BF16, isOutput=False)
    # wct[i, k, o] = wc[o, i, k//3, k%3]
    wct = nc.declare_dram_parameter("wct", [C, 9 * C], BF16, isOutput=False)
    # pcomb cols (all 128 rows): 0 bh, 1 bw, 2 s2, 3 b2 (= bc*s2+be2-m2*s2),
    # 4 bh/2 (tanh path); cols 6-9 (rows 0-7): s1/6, t1f/6, s1, t1f+3
    pcomb = nc.declare_dram_parameter("pcomb", [C, 10], F32, isOutput=False)
    outp = nc.declare_dram_parameter("out", [C, H, W], BF16, isOutput=True)

    with tile.TileContext(nc) as tc:
        with (
            tc.tile_pool(name="sing", bufs=1) as sing,
            tc.tile_pool(name="pp", bufs=2) as pp,
            tc.tile_pool(name="small", bufs=1) as small,
        ):
            xs = sing.tile([C, H * W], BF16)
            ug = sing.tile([C, HP, WP], BF16)
            s32b = sing.tile([C, 64, 4], F32)  # col-segment sums rows 64-127
            wtile = sing.tile([C, 512], BF16)  # zeros; warm-matmul fodder

            # Everything rides the sync HW ring: the gpsimd SWDGE ring has a
            # ~0.9us wakeup latency and its completion semaphores stall
            # behind trigger issuance on the GpSimd sequencer. The HW ring is
            # in-order per engine, so interleaving small weight loads between
            # x chunks preserves the chunk-chase sequencing, and wct's 288KB
            # queue behind all of x (no HBM contention mid-stream).
            XCH = [(0, 16), (16, 16), (32, 32), (64, 32), (96, 16), (112, 8), (120, 8)]
            w1ts_sb = sing.tile([C, 3, MIP], BF16)
            whw_sb = sing.tile([MIP, 2 * C], BF16)
            pc_sb = sing.tile([C, 10], F32)
            wct_sb = sing.tile([C, 9, C], BF16)
            side = {
                0: [(w1ts_sb, w1ts.rearrange("c (r m) -> c r m", r=3))],
            }
            # (Tried: last x chunks on the gpsimd SW ring to dodge the tail
            # stagger — regressed ~10us; bulk transfers on the two rings do
            # not overlap cleanly. Tiny side loads are fine there though.)
            for ci, (r0, nr) in enumerate(XCH):
                nc.sync.dma_start(
                    out=xs[:, ds(r0 * W, nr * W)],
                    in_=xp[:, ds(r0 * W, nr * W)],
                )
                for dst, src in side.get(ci, []):
                    nc.sync.dma_start(out=dst, in_=src)
            nc.sync.dma_start(out=wct_sb, in_=wct.rearrange("i (k o) -> i k o", k=9))
            # pcomb/whw ride the gpsimd ring: ~9KB total, needed only at
            # ~+14us, and keeping the sync ring at 9 triggers stays within
            # the DMA semaphore pool (no reuse-wait stalls on x triggers).
            nc.gpsimd.dma_start(out=pc_sb, in_=pcomb[:, :])
            nc.gpsimd.dma_start(out=whw_sb, in_=whw[:, :])
            wht_sb = whw_sb[:, 0:C]
            wwt_sb = whw_sb[:, C : 2 * C]
            p128_sb = pc_sb[:, 0:5]
            p8_sb = pc_sb[0:MIP, 6:10]

            # warm fodder first so the PE can start ramping ASAP
            nc.vector.memset(wtile, 0.0)
            # conv padding border of ug
            nc.vector.memset(ug[:, 0, :], 0.0)
            nc.vector.memset(ug[:, HP - 1, :], 0.0)
            nc.vector.memset(ug[:, 1 : HP - 1, 0:2], 0.0)
            nc.vector.memset(ug[:, 1 : HP - 1, WP - 2 : WP], 0.0)

            # ACT tables: preload the sigmoid set for the a_w / a_h(0-63)
            # activations (critical path); the compiler inserts ONE switch to
            # the silu set after the last sigmoid, landing in the ACT-idle
            # window before the first conv Silu. The mid-conv a_h(64-127)
            # activation uses Tanh, which the silu set also contains.
            dump = small.tile([C, 2], F32)
            nc.scalar.activation(dump, wtile[:, 0:2], AF.Sigmoid, bias=0.0, scale=1.0)

            with tc.tile_pool(name="psA", bufs=1, space="PSUM") as psA:
                ps_yh = psA.tile([MIP, 64], F32, tag="yh")
                ps_ah = psA.tile([C, H], F32, tag="ah")
                ps_aw = psA.tile([C, W], F32, tag="aw")
                ah_sb = small.tile([C, H], BF16)
                ah_t = small.tile([C, H], F32)  # raw tanh before affine

                def emit_tree(r0, nr):
                    # 32-col segment sums for rows [r0, r0+nr) (rows >= 64).
                    # DVE only: Pool elementwise ops halve concurrent PE/DVE
                    # throughput (SBUF contention), so Pool stays idle.
                    eng = nc.vector
                    xc = xs[:, ds(r0 * W, nr * W)].rearrange(
                        "p (y q s) -> p y q s", q=4, s=32
                    )
                    t1 = pp.tile([C, 16, 4, 16], BF16, tag="t1")
                    eng.tensor_add(t1[:, :nr], xc[:, :, :, 0:16], xc[:, :, :, 16:32])
                    t2 = pp.tile([C, 16, 4, 8], BF16, tag="t2")
                    eng.tensor_add(t2[:, :nr], t1[:, :nr, :, 0:8], t1[:, :nr, :, 8:16])
                    t3 = pp.tile([C, 16, 4, 4], BF16, tag="t3")
                    eng.tensor_add(t3[:, :nr], t2[:, :nr, :, 0:4], t2[:, :nr, :, 4:8])
                    t4 = pp.tile([C, 16, 4, 2], BF16, tag="t4")
                    eng.tensor_add(t4[:, :nr], t3[:, :nr, :, 0:2], t3[:, :nr, :, 2:4])
                    eng.tensor_add(
                        s32b[:, ds(r0 - 64, nr), :], t4[:, :nr, :, 0], t4[:, :nr, :, 1]
                    )

                def bn_hswish(src, dst, n, eng=None):
                    # dst = h_swish(s1*src + t1f) / 6 in THREE ops (the /6 is
                    # repaid by scaling the downstream wh/ww weights by 6):
                    # v = (u+3)/6, w = clamp(v,0,1), dst = (v-0.5)*w
                    eng = eng or nc.vector
                    v = pp.tile([MIP, n], F32, tag="bn_v")
                    eng.tensor_scalar(
                        out=v, in0=src, scalar1=p8_sb[:, 0:1],
                        scalar2=p8_sb[:, 1:2], op0=ALU.mult, op1=ALU.add,
                    )
                    w = pp.tile([MIP, n], F32, tag="bn_w")
                    eng.tensor_scalar(
                        out=w, in0=v, scalar1=0.0, scalar2=1.0,
                        op0=ALU.max, op1=ALU.min,
                    )
                    eng.scalar_tensor_tensor(
                        out=dst, in0=v, scalar=0.5, in1=w,
                        op0=ALU.subtract, op1=ALU.mult,
                    )

                def ah_mm_tanh(rlo, rhi, xh_sh):
                    # a_h = sigmoid(wh @ xh + bh) via 0.5+0.5*tanh(.5z+.5bh):
                    # runs mid-conv when the silu table is resident (the silu
                    # set contains Tanh, so no table switch is triggered)
                    nc.tensor.matmul(
                        ps_ah[:, ds(rlo, rhi - rlo)], wht_sb, xh_sh,
                        start=True, stop=True,
                    )
                    nc.scalar.activation(
                        ah_t[:, ds(rlo, rhi - rlo)], ps_ah[:, ds(rlo, rhi - rlo)],
                        AF.Tanh, bias=p128_sb[:, 4:5], scale=0.5,
                    )
                    nc.vector.tensor_scalar(
                        out=ah_sb[:, ds(rlo, rhi - rlo)],
                        in0=ah_t[:, ds(rlo, rhi - rlo)],
                        scalar1=0.5, scalar2=0.5, op0=ALU.mult, op1=ALU.add,
                    )

                def gate_rows(rlo, rhi):
                    # ug rows = x * a_h[c,y] * a_w[c,x], 4 rows per op pair:
                    # pass1 multiplies by a_h (stride-0 bcast along x, 1x DVE
                    # rate) — it only needs a_h, so the vector queue's bypass
                    # window precomputes the first tg tiles while the a_w
                    # chain is still running; pass2 (packed, 2x rate) applies
                    # a_w. ~30% cheaper than per-row scalar_tensor_tensor.
                    # leading groups are 2 rows: the first ~2 pass1 ops
                    # prefire into bn-chain bubbles (tg recycling caps it at
                    # 2), and smaller ops there halve the injected delay
                    if rlo == 0:
                        groups = [(y, 2) for y in range(0, 8, 2)]
                        groups += [(y, 4) for y in range(8, rhi, 4)]
                    else:
                        groups = [(y, 4) for y in range(rlo, rhi, 4)]
                    for y, g in groups:
                        tg = pp.tile([C, 4, W], BF16, tag="tg")
                        x4 = xs[:, ds(y * W, g * W)].rearrange(
                            "p (r x) -> p r x", r=g
                        )
                        ah_b = ah_sb[:, y : y + g, None].broadcast_to([C, g, W])
                        aw_b = aw_sb[:, None, :].broadcast_to([C, g, W])
                        nc.vector.tensor_mul(tg[:, :g], x4, ah_b)
                        nc.vector.tensor_mul(
                            ug[:, 1 + y : 1 + y + g, 2 : 2 + W], tg[:, :g], aw_b
                        )

                with tc.tile_pool(name="psW", bufs=1, space="PSUM") as psW:
                    ps_warm = psW.tile([C, 512], F32, tag="warm")
                    # x_w row-pool: 4-row matmuls with range-prescaled w1
                    # accumulate onto ONE (8, 4, W) psum bank
                    ps_xw = psW.tile([MIP, 4, W], F32, tag="xw")
                    # x_h col-pool for rows 0-63: per 16-row group, 8
                    # 16-col seg-matmuls (weights c1/c1/c2/c2/c3.. per seg)
                    # accumulate w1-projected partial sums
                    ps_xh0 = psW.tile([MIP, 64, 16], F32, tag="xh0")

                    def warm(n):
                        # dep-free warms: bridge PE gaps / hold the p-state
                        for _ in range(n):
                            nc.tensor.matmul(
                                ps_warm, wtile[:, 0:C], wtile,
                                start=True, stop=True,
                            )

                    def warm_tail(n):
                        # warms pinned on the LAST x chunk: fill the PE gap
                        # between the pool matmuls and the conv
                        for _ in range(n):
                            nc.tensor.matmul(
                                ps_warm, wtile[:, 0:C],
                                xs[:, ds(H * W - 512, 512)],
                                start=True, stop=True,
                            )

                    # ---- short PE ramp before chunk 0 lands
                    warm(2)

                    # ---- chunk-chasing ----
                    for r0, nr in XCH:
                        for g4 in range(r0, r0 + nr, 4):
                            nc.tensor.matmul(
                                ps_xw,
                                w1ts_sb[:, min(g4 // 32, 2), :],
                                xs[:, ds(g4 * W, 4 * W)],
                                start=(g4 == 0),
                                stop=(g4 == H - 4),
                                skip_group_check=True,
                            )
                        if r0 < 64:
                            # x_h seg-matmuls, 16-row psum slices
                            for b0 in range(r0, r0 + nr, 16):
                                nb = min(16, r0 + nr - b0)
                                xv = xs[:, ds(b0 * W, nb * W)].rearrange(
                                    "p (y q s) -> p y q s", q=8, s=16
                                )
                                for s in range(8):
                                    nc.tensor.matmul(
                                        ps_xh0[:, ds(b0, nb), :],
                                        w1ts_sb[:, min(s // 2, 2), :],
                                        xv[:, :, s, :],
                                        start=(s == 0),
                                        stop=(s == 7),
                                        skip_group_check=True,
                                    )
                        if r0 + nr == 64:
                            # rows 0-63 x_h: reduce psum -> yh directly (w1
                            # already applied), then bn + a_h matmul+sigmoid
                            yh0 = small.tile([MIP, 64], F32)
                            nc.vector.tensor_reduce(
                                out=yh0,
                                in_=ps_xh0,
                                axis=mybir.AxisListType.X,
                                op=ALU.add,
                            )
                            xhsA = pp.tile([MIP, 64], BF16, tag="xh_sh0")
                            bn_hswish(yh0, xhsA, 64)
                            nc.tensor.matmul(
                                ps_ah[:, 0:64], wht_sb, xhsA,
                                start=True, stop=True,
                            )
                            nc.scalar.activation(
                                ah_sb[:, 0:64], ps_ah[:, 0:64],
                                AF.Sigmoid, bias=p128_sb[:, 0:1], scale=1.0,
                            )

                    # ---- a_w chain: the critical path ----
                    # All-vector: trees 80-127 are deferred behind the first
                    # gates, so the vector queue is clear the moment the stop
                    # matmul lands. (Pool is NOT an option: its software ops
                    # cost ~2us for scalar-ptr / max-min, and it cannot read
                    # PSUM.)
                    xwp = small.tile([MIP, W], F32)
                    nc.vector.tensor_reduce(
                        out=xwp,
                        in_=ps_xw.rearrange("p j x -> p x j"),
                        axis=mybir.AxisListType.X,
                        op=ALU.add,
                    )
                    xw_s = small.tile([MIP, W], BF16)
                    bn_hswish(xwp, xw_s, W)
                    nc.tensor.matmul(ps_aw, wwt_sb, xw_s, start=True, stop=True)
                    aw_sb = small.tile([C, W], BF16)
                    nc.scalar.activation(
                        aw_sb, ps_aw, AF.Sigmoid, bias=p128_sb[:, 1:2], scale=1.0
                    )

                    # gates 0-63 unblock the conv ASAP; everything for rows
                    # 64-127 queues behind them (conv reaches row 64 only
                    # ~30us after it starts)
                    gate_rows(0, 64)

                    # rows 64-127 x_h path (deferred trees + combine): pooled
                    # -> yh matmul -> bn -> a_h matmul+tanh. Fully off the
                    # critical path (conv reaches row 64 ~35us later).
                    # xhpB uses integer weights 7/3/1; the stationary
                    # w1ts[2] = w1/128 then yields exactly
                    # w1 @ (7*S0 + 3*S1 + S2 + S3)/128 = w1 @ x_h-pool
                    emit_tree(64, 16)
                    emit_tree(80, 16)
                    emit_tree(96, 16)
                    emit_tree(112, 16)
                    tmpA = pp.tile([C, 64], F32, tag="tmpA")
                    nc.vector.tensor_add(tmpA, s32b[:, :, 2], s32b[:, :, 3])
                    m0 = pp.tile([C, 64], F32, tag="m0")
                    nc.vector.tensor_scalar_mul(m0, s32b[:, :, 0], 7.0)
                    m1 = pp.tile([C, 64], F32, tag="m1")
                    nc.vector.scalar_tensor_tensor(
                        out=m1, in0=s32b[:, :, 1], scalar=3.0, in1=m0,
                        op0=ALU.mult, op1=ALU.add,
                    )
                    xhpB = pp.tile([C, 64], BF16, tag="xhpB")
                    nc.vector.tensor_add(xhpB, tmpA, m1)

                    # bridge the post-pool PE idle until the conv starts;
                    # sized so the tails run dry just before the first conv
                    # matmul becomes ready (leftover tails would delay it)
                    warm_tail(14)

                # ---- 3x3 conv + BN2 + SiLU ----
                # The rows-64-127 a_h PE matmuls are spliced in after conv
                # block 10: their vector feeders complete behind gates 0-63,
                # and a stalled LDWEIGHTS earlier in the PE queue blocks all
                # later matmuls (no bypass once a stationary starts loading).
                # By block 11 the feeders are long ready, and gates 64-127
                # still land ~10us before conv block 16 consumes them.
                with (
                    tc.tile_pool(name="psB", bufs=5, space="PSUM") as psB,
                    tc.tile_pool(name="obp", bufs=4) as obp,
                ):
                    def conv_block(rb):
                        pso = psB.tile([C, 4, W], F32, tag="pso")
                        for k in range(9):
                            dy, dx = k // 3, k % 3
                            nc.tensor.matmul(
                                pso,
                                wct_sb[:, k, :],
                                ug[:, 4 * rb + dy : 4 * rb + dy + 4,
                                   1 + dx : 1 + dx + W],
                                start=(k == 0),
                                stop=(k == 8),
                            )
                        ob = obp.tile([C, 4, W], BF16, tag="ob")
                        nc.scalar.activation(
                            ob, pso, AF.Silu,
                            bias=p128_sb[:, 3:4], scale=p128_sb[:, 2:3],
                        )
                        nc.sync.dma_start(
                            out=outp[:, 4 * rb : 4 * rb + 4, :], in_=ob
                        )

                    for rb in range(12):
                        conv_block(rb)

                    nc.tensor.matmul(
                        ps_yh, w1ts_sb[:, 2, :], xhpB, start=True, stop=True
                    )
                    xhsB = pp.tile([MIP, 64], BF16, tag="xh_sh64")
                    bn_hswish(ps_yh, xhsB, 64)
                    ah_mm_tanh(64, 128, xhsB)
                    gate_rows(64, 128)

                    for rb in range(12, H // 4):
                        conv_block(rb)

    nc.compile()
    return nc


def prep_inputs(x, w1, b1, g1, be1, m1, v1, wh, bh, ww, bw, wc, bc, g2, be2, m2, v2):
    """Host-side prep: per-core input maps (weights replicated)."""
    bf = ml_dtypes.bfloat16
    N = x.shape[0]
    s1 = (g1 / np.sqrt(v1 + EPS)).astype(np.float64)
    t1f = s1 * b1 + be1 - m1 * s1
    # bn_hswish consts: v = (s1/6)*x + (t1f/6 + 0.5); cols 2-3 unused
    p8 = np.stack(
        [s1 / 6.0, t1f / 6.0 + 0.5, 0.0 * s1, 0.0 * s1], axis=1
    ).astype(np.float32)
    s2 = (g2 / np.sqrt(v2 + EPS)).astype(np.float64)
    b2 = bc * s2 + be2 - m2 * s2
    # pcomb cols: 0 bh, 1 bw (sigmoid path), 2 s2, 3 b2, 4 bh/2 (tanh
    # path); cols 6-9 rows 0-7: the p8 block
    pcomb = np.zeros((C, 10), np.float32)
    pcomb[:, 0:4] = np.stack([bh, bw, s2, b2], axis=1)
    pcomb[:, 4] = 0.5 * bh
    pcomb[0:MIP, 6:10] = p8
    cc = np.array([7.0 / 128, 3.0 / 128, 1.0 / 128])
    w1ts = np.stack([w1.T * c for c in cc], axis=1)              # (C, 3, MIP)
    shared = {
        "w1ts": np.ascontiguousarray(w1ts.reshape(C, 3 * MIP)).astype(bf),
        "whw": np.ascontiguousarray(
            6.0 * np.concatenate([wh.T, ww.T], axis=1)
        ).astype(bf),                                            # (MIP, 2C)
        "wct": np.ascontiguousarray(
            np.transpose(wc, (1, 2, 3, 0)).reshape(C, 9 * C)
        ).astype(bf),                                            # [i, (ky kx), o]
        "pcomb": pcomb,
    }
    in_maps = []
    for n in range(N):
        m = dict(shared)
        m["x"] = np.ascontiguousarray(x[n].reshape(C, H * W)).astype(bf)
        in_maps.append(m)
    return in_maps


def run(inputs, trace=False):
    if "nc" not in _CACHE:
        _CACHE["nc"] = build_nc()
    nc = _CACHE["nc"]
    in_maps = prep_inputs(**inputs)
    res = run_bass_kernel_spmd(nc, in_maps, core_ids=list(range(8)), trace=trace)
    out = np.stack([np.asarray(res.results[i]["out"]) for i in range(8)], axis=0)
    return out.astype(np.float32), res


def kernel(**inputs) -> np.ndarray:
    out, _ = run(inputs, trace=False)
    return out


# revision 47
# speedup vs baseline: 1.1781x; 1.1781x over previous
"""Trainium2 Bass kernel for CAConv2 (coordinate-attention + 3x3 conv block).

Shapes (hardcoded): x (8, 128, 128, 128) f32; data-parallel over batch,
one image per NeuronCore (8 cores).

Scheduling facts learned from HW traces / the cost model:
- PE streams conv matmuls at ~216ns/512 cols (~2.3 GHz effective) once
  warm; the 3x3 conv (32 blocks x 9 matmuls) is a ~63us floor and
  everything else must hide around it.
- Engines run their queues in order with a ~4-deep stalled-instruction
  bypass window: emission position is priority, dependencies are
  eligibility. A stalled matmul whose LDWEIGHTS has issued blocks ALL
  later matmuls, so PE matmuls with late feeders must be emitted at a
  queue position where their inputs are certainly ready (the B-path
  a_h matmuls are spliced between conv blocks 11 and 12).
- Pool (gpsimd) elementwise ops halve concurrent PE/DVE throughput
  (SBUF contention) and cannot touch PSUM; scalar-ptr / max-min Pool
  ops cost ~2us. Pool is left idle on purpose.
- DMA engines (16) sustain ~21 GB/s each (~300-340 GB/s aggregate); x
  (4MB bf16) needs ~13.5us of drain. All loads ride the sync HW ring
  (the gpsimd SW ring has ~1us wakeup and its completion semaphores
  stall behind trigger issuance); wct queues behind all of x on the
  same in-order ring. DMA triggers cost ~0.6-0.8us each on the issuing
  queue and completion semaphores fire ~1-2us after last data (engine
  stagger).
- ACT tables load per *set*: sigmoid set first for the critical a_w /
  a_h(0-63) activations; the single switch to the silu set lands in the
  ACT-idle window before the first conv Silu; the mid-conv a_h(64-127)
  sigmoid is computed as 0.5+0.5*tanh(z/2) (Tanh is in the silu set).
- Cross-engine dependency hops cost ~0.3-1us; run-to-run variance is
  ~+-1.5us (DMA ring behavior).
"""

import numpy as np
import ml_dtypes

import concourse.bacc as bacc
import concourse.tile as tile
from concourse import mybir
from concourse.bass import ds
from concourse.bass_utils import run_bass_kernel_spmd

BF16 = mybir.dt.bfloat16
F32 = mybir.dt.float32
C, H, W, MIP = 128, 128, 128, 8
WP = W + 4  # padded width: cols [2, 130) hold data, 0/1 and 130/131 are zero
HP = H + 2  # padded height: rows [1, 129) hold data
EPS = 1e-5
AF = mybir.ActivationFunctionType
ALU = mybir.AluOpType

_CACHE = {}


def build_nc():
    nc = bacc.Bacc(num_swdge_queues=1)
    xp = nc.declare_dram_parameter("x", [C, H * W], BF16, isOutput=False)
    w1ts = nc.declare_dram_parameter("w1ts", [C, 3 * MIP], BF16, isOutput=False)
    # wht | wwt packed side by side (one DMA trigger)
    whw = nc.declare_dram_parameter("whw", [MIP, 2 * C], BF16, isOutput=False)
    # wct[i, k, o] = wc[o, i, k//3, k%3]
    wct = nc.declare_dram_parameter("wct", [C, 9 * C], BF16, isOutput=False)
    # pcomb cols (all 128 rows): 0 bh, 1 bw, 2 s2, 3 b2 (= bc*s2+be2-m2*s2),
    # 4 bh/2 (tanh path); cols 6-9 (rows 0-7): s1/6, t1f/6, s1, t1f+3
    pcomb = nc.declare_dram_parameter("pcomb", [C, 10], F32, isOutput=False)
    outp = nc.declare_dram_parameter("out", [C, H, W], BF16, isOutput=True)

    with tile.TileContext(nc) as tc:
        with (
            tc.tile_pool(name="sing", bufs=1) as sing,
            tc.tile_pool(name="pp", bufs=2) as pp,
            tc.tile_pool(name="small", bufs=1) as small,
        ):
            xs = sing.tile([C, H * W], BF16)
            ug = sing.tile([C, HP, WP], BF16)
            s32b = sing.tile([C, 64, 4], F32)  # col-segment sums rows 64-127
            wtile = sing.tile([C, 512], BF16)  # zeros; warm-matmul fodder

            # Everything rides the sync HW ring: the gpsimd SWDGE ring has a
            # ~0.9us wakeup latency and its completion semaphores stall
            # behind trigger issuance on the GpSimd sequencer. The HW ring is
            # in-order per engine, so interleaving small weight loads between
            # x chunks preserves the chunk-chase sequencing, and wct's 288KB
            # queue behind all of x (no HBM contention mid-stream).
            XCH = [(0, 16), (16, 16), (32, 32), (64, 32), (96, 16), (112, 8), (120, 8)]
            w1ts_sb = sing.tile([C, 3, MIP], BF16)
            whw_sb = sing.tile([MIP, 2 * C], BF16)
            pc_sb = sing.tile([C, 10], F32)
            wct_sb = sing.tile([C, 9, C], BF16)
            side = {
                0: [(w1ts_sb, w1ts.rearrange("c (r m) -> c r m", r=3))],
            }
            # (Tried: last x chunks on the gpsimd SW ring to dodge the tail
            # stagger — regressed ~10us; bulk transfers on the two rings do
            # not overlap cleanly. Tiny side loads are fine there though.)
            for ci, (r0, nr) in enumerate(XCH):
                nc.sync.dma_start(
                    out=xs[:, ds(r0 * W, nr * W)],
                    in_=xp[:, ds(r0 * W, nr * W)],
                )
                for dst, src in side.get(ci, []):
                    nc.sync.dma_start(out=dst, in_=src)
            nc.sync.dma_start(out=wct_sb, in_=wct.rearrange("i (k o) -> i k o", k=9))
            # pcomb/whw ride the gpsimd ring: ~9KB total, needed only at
            # ~+14us, and keeping the sync ring at 9 triggers stays within
            # the DMA semaphore pool (no reuse-wait stalls on x triggers).
            nc.gpsimd.dma_start(out=pc_sb, in_=pcomb[:, :])
            nc.gpsimd.dma_start(out=whw_sb, in_=whw[:, :])
            wht_sb = whw_sb[:, 0:C]
            wwt_sb = whw_sb[:, C : 2 * C]
            p128_sb = pc_sb[:, 0:5]
            p8_sb = pc_sb[0:MIP, 6:10]

            # warm fodder first so the PE can start ramping ASAP
            nc.vector.memset(wtile, 0.0)
            # conv padding border of ug
            nc.vector.memset(ug[:, 0, :], 0.0)
            nc.vector.memset(ug[:, HP - 1, :], 0.0)
            nc.vector.memset(ug[:, 1 : HP - 1, 0:2], 0.0)
            nc.vector.memset(ug[:, 1 : HP - 1, WP - 2 : WP], 0.0)

            # ACT tables: preload the sigmoid set for the a_w / a_h(0-63)
            # activations (critical path); the compiler inserts ONE switch to
            # the silu set after the last sigmoid, landing in the ACT-idle
            # window before the first conv Silu. The mid-conv a_h(64-127)
            # activation uses Tanh, which the silu set also contains.
            dump = small.tile([C, 2], F32)
            nc.scalar.activation(dump, wtile[:, 0:2], AF.Sigmoid, bias=0.0, scale=1.0)

            with tc.tile_pool(name="psA", bufs=1, space="PSUM") as psA:
                ps_yh = psA.tile([MIP, 64], F32, tag="yh")
                ps_ah = psA.tile([C, H], F32, tag="ah")
                ps_aw = psA.tile([C, W], F32, tag="aw")
                ah_sb = small.tile([C, H], BF16)
                ah_t = small.tile([C, H], F32)  # raw tanh before affine

                def emit_tree(r0, nr):
                    # 32-col segment sums for rows [r0, r0+nr) (rows >= 64).
                    # DVE only: Pool elementwise ops halve concurrent PE/DVE
                    # throughput (SBUF contention), so Pool stays idle.
                    eng = nc.vector
                    xc = xs[:, ds(r0 * W, nr * W)].rearrange(
                        "p (y q s) -> p y q s", q=4, s=32
                    )
                    t1 = pp.tile([C, 16, 4, 16], BF16, tag="t1")
                    eng.tensor_add(t1[:, :nr], xc[:, :, :, 0:16], xc[:, :, :, 16:32])
                    t2 = pp.tile([C, 16, 4, 8], BF16, tag="t2")
                    eng.tensor_add(t2[:, :nr], t1[:, :nr, :, 0:8], t1[:, :nr, :, 8:16])
                    t3 = pp.tile([C, 16, 4, 4], BF16, tag="t3")
                    eng.tensor_add(t3[:, :nr], t2[:, :nr, :, 0:4], t2[:, :nr, :, 4:8])
                    t4 = pp.tile([C, 16, 4, 2], BF16, tag="t4")
                    eng.tensor_add(t4[:, :nr], t3[:, :nr, :, 0:2], t3[:, :nr, :, 2:4])
                    eng.tensor_add(
                        s32b[:, ds(r0 - 64, nr), :], t4[:, :nr, :, 0], t4[:, :nr, :, 1]
                    )

                def bn_hswish(src, dst, n, eng=None):
                    # dst = h_swish(s1*src + t1f) / 6 in THREE ops (the /6 is
                    # repaid by scaling the downstream wh/ww weights by 6):
                    # v = (u+3)/6, w = clamp(v,0,1), dst = (v-0.5)*w
                    eng = eng or nc.vector
                    v = pp.tile([MIP, n], F32, tag="bn_v")
                    eng.tensor_scalar(
                        out=v, in0=src, scalar1=p8_sb[:, 0:1],
                        scalar2=p8_sb[:, 1:2], op0=ALU.mult, op1=ALU.add,
                    )
                    w = pp.tile([MIP, n], F32, tag="bn_w")
                    eng.tensor_scalar(
                        out=w, in0=v, scalar1=0.0, scalar2=1.0,
                        op0=ALU.max, op1=ALU.min,
                    )
                    eng.scalar_tensor_tensor(
                        out=dst, in0=v, scalar=0.5, in1=w,
                        op0=ALU.subtract, op1=ALU.mult,
                    )

                def ah_mm_tanh(rlo, rhi, xh_sh):
                    # a_h = sigmoid(wh @ xh + bh) via 0.5+0.5*tanh(.5z+.5bh):
                    # runs mid-conv when the silu table is resident (the silu
                    # set contains Tanh, so no table switch is triggered)
                    nc.tensor.matmul(
                        ps_ah[:, ds(rlo, rhi - rlo)], wht_sb, xh_sh,
                        start=True, stop=True,
                    )
                    nc.scalar.activation(
                        ah_t[:, ds(rlo, rhi - rlo)], ps_ah[:, ds(rlo, rhi - rlo)],
                        AF.Tanh, bias=p128_sb[:, 4:5], scale=0.5,
                    )
                    nc.vector.tensor_scalar(
                        out=ah_sb[:, ds(rlo, rhi - rlo)],
                        in0=ah_t[:, ds(rlo, rhi - rlo)],
                        scalar1=0.5, scalar2=0.5, op0=ALU.mult, op1=ALU.add,
                    )

                def gate_rows(rlo, rhi):
                    # ug rows = x * a_h[c,y] * a_w[c,x], 4 rows per op pair:
                    # pass1 multiplies by a_h (stride-0 bcast along x, 1x DVE
                    # rate) — it only needs a_h, so the vector queue's bypass
                    # window precomputes the first tg tiles while the a_w
                    # chain is still running; pass2 (packed, 2x rate) applies
                    # a_w. ~30% cheaper than per-row scalar_tensor_tensor.
                    aw_b = aw_sb[:, None, :].broadcast_to([C, 4, W])
                    for y in range(rlo, rhi, 4):
                        tg = pp.tile([C, 4, W], BF16, tag="tg")
                        x4 = xs[:, ds(y * W, 4 * W)].rearrange(
                            "p (r x) -> p r x", r=4
                        )
                        ah_b = ah_sb[:, y : y + 4, None].broadcast_to([C, 4, W])
                        nc.vector.tensor_mul(tg, x4, ah_b)
                        nc.vector.tensor_mul(
                            ug[:, 1 + y : 5 + y, 2 : 2 + W], tg, aw_b
                        )

                with tc.tile_pool(name="psW", bufs=1, space="PSUM") as psW:
                    ps_warm = psW.tile([C, 512], F32, tag="warm")
                    # x_w row-pool: 4-row matmuls with range-prescaled w1
                    # accumulate onto ONE (8, 4, W) psum bank
                    ps_xw = psW.tile([MIP, 4, W], F32, tag="xw")
                    # x_h col-pool for rows 0-63: per 16-row group, 8
                    # 16-col seg-matmuls (weights c1/c1/c2/c2/c3.. per seg)
                    # accumulate w1-projected partial sums
                    ps_xh0 = psW.tile([MIP, 64, 16], F32, tag="xh0")

                    def warm(n):
                        # dep-free warms: bridge PE gaps / hold the p-state
                        for _ in range(n):
                            nc.tensor.matmul(
                                ps_warm, wtile[:, 0:C], wtile,
                                start=True, stop=True,
                            )

                    def warm_tail(n):
                        # warms pinned on the LAST x chunk: fill the PE gap
                        # between the pool matmuls and the conv
                        for _ in range(n):
                            nc.tensor.matmul(
                                ps_warm, wtile[:, 0:C],
                                xs[:, ds(H * W - 512, 512)],
                                start=True, stop=True,
                            )

                    # ---- short PE ramp before chunk 0 lands
                    warm(2)

                    # ---- chunk-chasing ----
                    for r0, nr in XCH:
                        for g4 in range(r0, r0 + nr, 4):
                            nc.tensor.matmul(
                                ps_xw,
                                w1ts_sb[:, min(g4 // 32, 2), :],
                                xs[:, ds(g4 * W, 4 * W)],
                                start=(g4 == 0),
                                stop=(g4 == H - 4),
                                skip_group_check=True,
                            )
                        if r0 < 64:
                            # x_h seg-matmuls, 16-row psum slices
                            for b0 in range(r0, r0 + nr, 16):
                                nb = min(16, r0 + nr - b0)
                                xv = xs[:, ds(b0 * W, nb * W)].rearrange(
                                    "p (y q s) -> p y q s", q=8, s=16
                                )
                                for s in range(8):
                                    nc.tensor.matmul(
                                        ps_xh0[:, ds(b0, nb), :],
                                        w1ts_sb[:, min(s // 2, 2), :],
                                        xv[:, :, s, :],
                                        start=(s == 0),
                                        stop=(s == 7),
                                        skip_group_check=True,
                                    )
                        if r0 + nr == 64:
                            # rows 0-63 x_h: reduce psum -> yh directly (w1
                            # already applied), then bn + a_h matmul+sigmoid
                            yh0 = small.tile([MIP, 64], F32)
                            nc.vector.tensor_reduce(
                                out=yh0,
                                in_=ps_xh0,
                                axis=mybir.AxisListType.X,
                                op=ALU.add,
                            )
                            xhsA = pp.tile([MIP, 64], BF16, tag="xh_sh0")
                            bn_hswish(yh0, xhsA, 64)
                            nc.tensor.matmul(
                                ps_ah[:, 0:64], wht_sb, xhsA,
                                start=True, stop=True,
                            )
                            nc.scalar.activation(
                                ah_sb[:, 0:64], ps_ah[:, 0:64],
                                AF.Sigmoid, bias=p128_sb[:, 0:1], scale=1.0,
                            )

                    # ---- a_w chain: the critical path ----
                    # All-vector: trees 80-127 are deferred behind the first
                    # gates, so the vector queue is clear the moment the stop
                    # matmul lands. (Pool is NOT an option: its software ops
                    # cost ~2us for scalar-ptr / max-min, and it cannot read
                    # PSUM.)
                    xwp = small.tile([MIP, W], F32)
                    nc.vector.tensor_reduce(
                        out=xwp,
                        in_=ps_xw.rearrange("p j x -> p x j"),
                        axis=mybir.AxisListType.X,
                        op=ALU.add,
                    )
                    xw_s = small.tile([MIP, W], BF16)
                    bn_hswish(xwp, xw_s, W)
                    nc.tensor.matmul(ps_aw, wwt_sb, xw_s, start=True, stop=True)
                    aw_sb = small.tile([C, W], BF16)
                    nc.scalar.activation(
                        aw_sb, ps_aw, AF.Sigmoid, bias=p128_sb[:, 1:2], scale=1.0
                    )

                    # gates 0-63 unblock the conv ASAP; everything for rows
                    # 64-127 queues behind them (conv reaches row 64 only
                    # ~30us after it starts)
                    gate_rows(0, 64)

                    # rows 64-127 x_h path (deferred trees + combine): pooled
                    # -> yh matmul -> bn -> a_h matmul+tanh. Fully off the
                    # critical path (conv reaches row 64 ~35us later).
                    # xhpB uses integer weights 7/3/1; the stationary
                    # w1ts[2] = w1/128 then yields exactly
                    # w1 @ (7*S0 + 3*S1 + S2 + S3)/128 = w1 @ x_h-pool
                    emit_tree(64, 16)
                    emit_tree(80, 16)
                    emit_tree(96, 16)
                    emit_tree(112, 16)
                    tmpA = pp.tile([C, 64], F32, tag="tmpA")
                    nc.vector.tensor_add(tmpA, s32b[:, :, 2], s32b[:, :, 3])
                    m0 = pp.tile([C, 64], F32, tag="m0")
                    nc.vector.tensor_scalar_mul(m0, s32b[:, :, 0], 7.0)
                    m1 = pp.tile([C, 64], F32, tag="m1")
                    nc.vector.scalar_tensor_tensor(
                        out=m1, in0=s32b[:, :, 1], scalar=3.0, in1=m0,
                        op0=ALU.mult, op1=ALU.add,
                    )
                    xhpB = pp.tile([C, 64], BF16, tag="xhpB")
                    nc.vector.tensor_add(xhpB, tmpA, m1)

                    # bridge the post-pool PE idle until the conv starts;
                    # sized so the tails run dry just before the first conv
                    # matmul becomes ready (leftover tails would delay it)
                    warm_tail(14)

                # ---- 3x3 conv + BN2 + SiLU ----
                # The rows-64-127 a_h PE matmuls are spliced in after conv
                # block 10: their vector feeders complete behind gates 0-63,
                # and a stalled LDWEIGHTS earlier in the PE queue blocks all
                # later matmuls (no bypass once a stationary starts loading).
                # By block 11 the feeders are long ready, and gates 64-127
                # still land ~10us before conv block 16 consumes them.
                with (
                    tc.tile_pool(name="psB", bufs=5, space="PSUM") as psB,
                    tc.tile_pool(name="obp", bufs=4) as obp,
                ):
                    def conv_block(rb):
                        pso = psB.tile([C, 4, W], F32, tag="pso")
                        for k in range(9):
                            dy, dx = k // 3, k % 3
                            nc.tensor.matmul(
                                pso,
                                wct_sb[:, k, :],
                                ug[:, 4 * rb + dy : 4 * rb + dy + 4,
                                   1 + dx : 1 + dx + W],
                                start=(k == 0),
                                stop=(k == 8),
                            )
                        ob = obp.tile([C, 4, W], BF16, tag="ob")
                        nc.scalar.activation(
                            ob, pso, AF.Silu,
                            bias=p128_sb[:, 3:4], scale=p128_sb[:, 2:3],
                        )
                        nc.sync.dma_start(
                            out=outp[:, 4 * rb : 4 * rb + 4, :], in_=ob
                        )

                    for rb in range(12):
                        conv_block(rb)

                    nc.tensor.matmul(
                        ps_yh, w1ts_sb[:, 2, :], xhpB, start=True, stop=True
                    )
                    xhsB = pp.tile([MIP, 64], BF16, tag="xh_sh64")
                    bn_hswish(ps_yh, xhsB, 64)
                    ah_mm_tanh(64, 128, xhsB)
                    gate_rows(64, 128)

                    for rb in range(12, H // 4):
                        conv_block(rb)

    nc.compile()
    return nc


def prep_inputs(x, w1, b1, g1, be1, m1, v1, wh, bh, ww, bw, wc, bc, g2, be2, m2, v2):
    """Host-side prep: per-core input maps (weights replicated)."""
    bf = ml_dtypes.bfloat16
    N = x.shape[0]
    s1 = (g1 / np.sqrt(v1 + EPS)).astype(np.float64)
    t1f = s1 * b1 + be1 - m1 * s1
    # bn_hswish consts: v = (s1/6)*x + (t1f/6 + 0.5); cols 2-3 unused
    p8 = np.stack(
        [s1 / 6.0, t1f / 6.0 + 0.5, 0.0 * s1, 0.0 * s1], axis=1
    ).astype(np.float32)
    s2 = (g2 / np.sqrt(v2 + EPS)).astype(np.float64)
    b2 = bc * s2 + be2 - m2 * s2
    # pcomb cols: 0 bh, 1 bw (sigmoid path), 2 s2, 3 b2, 4 bh/2 (tanh
    # path); cols 6-9 rows 0-7: the p8 block
    pcomb = np.zeros((C, 10), np.float32)
    pcomb[:, 0:4] = np.stack([bh, bw, s2, b2], axis=1)
    pcomb[:, 4] = 0.5 * bh
    pcomb[0:MIP, 6:10] = p8
    cc = np.array([7.0 / 128, 3.0 / 128, 1.0 / 128])
    w1ts = np.stack([w1.T * c for c in cc], axis=1)              # (C, 3, MIP)
    shared = {
        "w1ts": np.ascontiguousarray(w1ts.reshape(C, 3 * MIP)).astype(bf),
        "whw": np.ascontiguousarray(
            6.0 * np.concatenate([wh.T, ww.T], axis=1)
        ).astype(bf),                                            # (MIP, 2C)
        "wct": np.ascontiguousarray(
            np.transpose(wc, (1, 2, 3, 0)).reshape(C, 9 * C)
        ).astype(bf),                                            # [i, (ky kx), o]
        "pcomb": pcomb,
    }
    in_maps = []
    for n in range(N):
        m = dict(shared)
        m["x"] = np.ascontiguousarray(x[n].reshape(C, H * W)).astype(bf)
        in_maps.append(m)
    return in_maps


def run(inputs, trace=False):
    if "nc" not in _CACHE:
        _CACHE["nc"] = build_nc()
    nc = _CACHE["nc"]
    in_maps = prep_inputs(**inputs)
    res = run_bass_kernel_spmd(nc, in_maps, core_ids=list(range(8)), trace=trace)
    out = np.stack([np.asarray(res.results[i]["out"]) for i in range(8)], axis=0)
    return out.astype(np.float32), res


def kernel(**inputs) -> np.ndarray:
    out, _ = run(inputs, trace=False)
    return out


# revision 49
# speedup vs baseline: 1.1782x; 1.0001x over previous
"""Trainium2 Bass kernel for CAConv2 (coordinate-attention + 3x3 conv block).

Shapes (hardcoded): x (8, 128, 128, 128) f32; data-parallel over batch,
one image per NeuronCore (8 cores).

Scheduling facts learned from HW traces / the cost model:
- PE streams conv matmuls at ~216ns/512 cols (~2.3 GHz effective) once
  warm; the 3x3 conv (32 blocks x 9 matmuls) is a ~63us floor and
  everything else must hide around it.
- Engines run their queues in order with a ~4-deep stalled-instruction
  bypass window: emission position is priority, dependencies are
  eligibility. A stalled matmul whose LDWEIGHTS has issued blocks ALL
  later matmuls, so PE matmuls with late feeders must be emitted at a
  queue position where their inputs are certainly ready (the B-path
  a_h matmuls are spliced between conv blocks 11 and 12).
- Pool (gpsimd) elementwise ops halve concurrent PE/DVE throughput
  (SBUF contention) and cannot touch PSUM; scalar-ptr / max-min Pool
  ops cost ~2us. Pool is left idle on purpose.
- DMA engines (16) sustain ~21 GB/s each (~300-340 GB/s aggregate); x
  (4MB bf16) needs ~13.5us of drain. All loads ride the sync HW ring
  (the gpsimd SW ring has ~1us wakeup and its completion semaphores
  stall behind trigger issuance); wct queues behind all of x on the
  same in-order ring. DMA triggers cost ~0.6-0.8us each on the issuing
  queue and completion semaphores fire ~1-2us after last data (engine
  stagger).
- ACT tables load per *set*: sigmoid set first for the critical a_w /
  a_h(0-63) activations; the single switch to the silu set lands in the
  ACT-idle window before the first conv Silu; the mid-conv a_h(64-127)
  sigmoid is computed as 0.5+0.5*tanh(z/2) (Tanh is in the silu set).
- Cross-engine dependency hops cost ~0.3-1us; run-to-run variance is
  ~+-1.5us (DMA ring behavior).
"""

import numpy as np
import ml_dtypes

import concourse.bacc as bacc
import concourse.tile as tile
from concourse import mybir
from concourse.bass import ds
from concourse.bass_utils import run_bass_kernel_spmd

BF16 = mybir.dt.bfloat16
F32 = mybir.dt.float32
C, H, W, MIP = 128, 128, 128, 8
WP = W + 4  # padded width: cols [2, 130) hold data, 0/1 and 130/131 are zero
HP = H + 2  # padded height: rows [1, 129) hold data
EPS = 1e-5
AF = mybir.ActivationFunctionType
ALU = mybir.AluOpType

_CACHE = {}


def build_nc():
    nc = bacc.Bacc(num_swdge_queues=1)
    xp = nc.declare_dram_parameter("x", [C, H * W], BF16, isOutput=False)
    w1ts = nc.declare_dram_parameter("w1ts", [C, 3 * MIP], BF16, isOutput=False)
    # wht | wwt packed side by side (one DMA trigger)
    whw = nc.declare_dram_parameter("whw", [MIP, 2 * C], BF16, isOutput=False)
    # wct[i, k, o] = wc[o, i, k//3, k%3]
    wct = nc.declare_dram_parameter("wct", [C, 9 * C], BF16, isOutput=False)
    # pcomb cols (all 128 rows): 0 bh, 1 bw, 2 s2, 3 b2 (= bc*s2+be2-m2*s2),
    # 4 bh/2 (tanh path); cols 6-9 (rows 0-7): s1/6, t1f/6, s1, t1f+3
    pcomb = nc.declare_dram_parameter("pcomb", [C, 10], F32, isOutput=False)
    outp = nc.declare_dram_parameter("out", [C, H, W], BF16, isOutput=True)

    with tile.TileContext(nc) as tc:
        with (
            tc.tile_pool(name="sing", bufs=1) as sing,
            tc.tile_pool(name="pp", bufs=2) as pp,
            # tg gets bufs=1 on purpose: DVE is serial so strict
            # pass1/pass2 alternation costs nothing, but it caps the
            # early-ready pass1 ops that can jump into the a_w bn chain's
            # semaphore gaps at ONE (each injection costs ~0.6us of
            # critical path)
            tc.tile_pool(name="tgp", bufs=1) as tgp,
            tc.tile_pool(name="small", bufs=1) as small,
        ):
            xs = sing.tile([C, H * W], BF16)
            ug = sing.tile([C, HP, WP], BF16)
            s32b = sing.tile([C, 64, 4], F32)  # col-segment sums rows 64-127
            wtile = sing.tile([C, 512], BF16)  # zeros; warm-matmul fodder

            # Everything rides the sync HW ring: the gpsimd SWDGE ring has a
            # ~0.9us wakeup latency and its completion semaphores stall
            # behind trigger issuance on the GpSimd sequencer. The HW ring is
            # in-order per engine, so interleaving small weight loads between
            # x chunks preserves the chunk-chase sequencing, and wct's 288KB
            # queue behind all of x (no HBM contention mid-stream).
            XCH = [(0, 16), (16, 16), (32, 32), (64, 32), (96, 16), (112, 8), (120, 8)]
            w1ts_sb = sing.tile([C, 3, MIP], BF16)
            whw_sb = sing.tile([MIP, 2 * C], BF16)
            pc_sb = sing.tile([C, 10], F32)
            wct_sb = sing.tile([C, 9, C], BF16)
            side = {
                0: [(w1ts_sb, w1ts.rearrange("c (r m) -> c r m", r=3))],
            }
            # (Tried: last x chunks on the gpsimd SW ring to dodge the tail
            # stagger — regressed ~10us; bulk transfers on the two rings do
            # not overlap cleanly. Tiny side loads are fine there though.)
            for ci, (r0, nr) in enumerate(XCH):
                nc.sync.dma_start(
                    out=xs[:, ds(r0 * W, nr * W)],
                    in_=xp[:, ds(r0 * W, nr * W)],
                )
                for dst, src in side.get(ci, []):
                    nc.sync.dma_start(out=dst, in_=src)
            nc.sync.dma_start(out=wct_sb, in_=wct.rearrange("i (k o) -> i k o", k=9))
            # pcomb/whw ride the gpsimd ring: ~9KB total, needed only at
            # ~+14us, and keeping the sync ring at 9 triggers stays within
            # the DMA semaphore pool (no reuse-wait stalls on x triggers).
            nc.gpsimd.dma_start(out=pc_sb, in_=pcomb[:, :])
            nc.gpsimd.dma_start(out=whw_sb, in_=whw[:, :])
            wht_sb = whw_sb[:, 0:C]
            wwt_sb = whw_sb[:, C : 2 * C]
            p128_sb = pc_sb[:, 0:5]
            p8_sb = pc_sb[0:MIP, 6:10]

            # warm fodder first so the PE can start ramping ASAP
            nc.vector.memset(wtile, 0.0)
            # conv padding border of ug
            nc.vector.memset(ug[:, 0, :], 0.0)
            nc.vector.memset(ug[:, HP - 1, :], 0.0)
            nc.vector.memset(ug[:, 1 : HP - 1, 0:2], 0.0)
            nc.vector.memset(ug[:, 1 : HP - 1, WP - 2 : WP], 0.0)

            # ACT tables: preload the sigmoid set for the a_w / a_h(0-63)
            # activations (critical path); the compiler inserts ONE switch to
            # the silu set after the last sigmoid, landing in the ACT-idle
            # window before the first conv Silu. The mid-conv a_h(64-127)
            # activation uses Tanh, which the silu set also contains.
            dump = small.tile([C, 2], F32)
            nc.scalar.activation(dump, wtile[:, 0:2], AF.Sigmoid, bias=0.0, scale=1.0)

            with tc.tile_pool(name="psA", bufs=1, space="PSUM") as psA:
                ps_yh = psA.tile([MIP, 64], F32, tag="yh")
                ps_ah = psA.tile([C, H], F32, tag="ah")
                ps_aw = psA.tile([C, W], F32, tag="aw")
                ah_sb = small.tile([C, H], BF16)
                ah_t = small.tile([C, H], F32)  # raw tanh before affine

                def emit_tree(r0, nr):
                    # 32-col segment sums for rows [r0, r0+nr) (rows >= 64).
                    # DVE only: Pool elementwise ops halve concurrent PE/DVE
                    # throughput (SBUF contention), so Pool stays idle.
                    eng = nc.vector
                    xc = xs[:, ds(r0 * W, nr * W)].rearrange(
                        "p (y q s) -> p y q s", q=4, s=32
                    )
                    t1 = pp.tile([C, 16, 4, 16], BF16, tag="t1")
                    eng.tensor_add(t1[:, :nr], xc[:, :, :, 0:16], xc[:, :, :, 16:32])
                    t2 = pp.tile([C, 16, 4, 8], BF16, tag="t2")
                    eng.tensor_add(t2[:, :nr], t1[:, :nr, :, 0:8], t1[:, :nr, :, 8:16])
                    t3 = pp.tile([C, 16, 4, 4], BF16, tag="t3")
                    eng.tensor_add(t3[:, :nr], t2[:, :nr, :, 0:4], t2[:, :nr, :, 4:8])
                    t4 = pp.tile([C, 16, 4, 2], BF16, tag="t4")
                    eng.tensor_add(t4[:, :nr], t3[:, :nr, :, 0:2], t3[:, :nr, :, 2:4])
                    eng.tensor_add(
                        s32b[:, ds(r0 - 64, nr), :], t4[:, :nr, :, 0], t4[:, :nr, :, 1]
                    )

                def bn_hswish(src, dst, n, eng=None):
                    # dst = h_swish(s1*src + t1f) / 6 in THREE ops (the /6 is
                    # repaid by scaling the downstream wh/ww weights by 6):
                    # v = (u+3)/6, w = clamp(v,0,1), dst = (v-0.5)*w
                    eng = eng or nc.vector
                    v = pp.tile([MIP, n], F32, tag="bn_v")
                    eng.tensor_scalar(
                        out=v, in0=src, scalar1=p8_sb[:, 0:1],
                        scalar2=p8_sb[:, 1:2], op0=ALU.mult, op1=ALU.add,
                    )
                    w = pp.tile([MIP, n], F32, tag="bn_w")
                    eng.tensor_scalar(
                        out=w, in0=v, scalar1=0.0, scalar2=1.0,
                        op0=ALU.max, op1=ALU.min,
                    )
                    eng.scalar_tensor_tensor(
                        out=dst, in0=v, scalar=0.5, in1=w,
                        op0=ALU.subtract, op1=ALU.mult,
                    )

                def ah_mm_tanh(rlo, rhi, xh_sh):
                    # a_h = sigmoid(wh @ xh + bh) via 0.5+0.5*tanh(.5z+.5bh):
                    # runs mid-conv when the silu table is resident (the silu
                    # set contains Tanh, so no table switch is triggered)
                    nc.tensor.matmul(
                        ps_ah[:, ds(rlo, rhi - rlo)], wht_sb, xh_sh,
                        start=True, stop=True,
                    )
                    nc.scalar.activation(
                        ah_t[:, ds(rlo, rhi - rlo)], ps_ah[:, ds(rlo, rhi - rlo)],
                        AF.Tanh, bias=p128_sb[:, 4:5], scale=0.5,
                    )
                    nc.vector.tensor_scalar(
                        out=ah_sb[:, ds(rlo, rhi - rlo)],
                        in0=ah_t[:, ds(rlo, rhi - rlo)],
                        scalar1=0.5, scalar2=0.5, op0=ALU.mult, op1=ALU.add,
                    )

                def gate_rows(rlo, rhi):
                    # ug rows = x * a_h[c,y] * a_w[c,x], 4 rows per op pair:
                    # pass1 multiplies by a_h (stride-0 bcast along x, 1x DVE
                    # rate) — it only needs a_h, so the vector queue's bypass
                    # window precomputes the first tg tiles while the a_w
                    # chain is still running; pass2 (packed, 2x rate) applies
                    # a_w. ~30% cheaper than per-row scalar_tensor_tensor.
                    aw_b = aw_sb[:, None, :].broadcast_to([C, 4, W])
                    for y in range(rlo, rhi, 4):
                        tg = tgp.tile([C, 4, W], BF16, tag="tg")
                        x4 = xs[:, ds(y * W, 4 * W)].rearrange(
                            "p (r x) -> p r x", r=4
                        )
                        ah_b = ah_sb[:, y : y + 4, None].broadcast_to([C, 4, W])
                        nc.vector.tensor_mul(tg, x4, ah_b)
                        nc.vector.tensor_mul(
                            ug[:, 1 + y : 5 + y, 2 : 2 + W], tg, aw_b
                        )

                with tc.tile_pool(name="psW", bufs=1, space="PSUM") as psW:
                    ps_warm = psW.tile([C, 512], F32, tag="warm")
                    # x_w row-pool: 4-row matmuls with range-prescaled w1
                    # accumulate onto ONE (8, 4, W) psum bank
                    ps_xw = psW.tile([MIP, 4, W], F32, tag="xw")
                    # x_h col-pool for rows 0-63: per 16-row group, 8
                    # 16-col seg-matmuls (weights c1/c1/c2/c2/c3.. per seg)
                    # accumulate w1-projected partial sums
                    ps_xh0 = psW.tile([MIP, 64, 16], F32, tag="xh0")

                    def warm(n):
                        # dep-free warms: bridge PE gaps / hold the p-state
                        for _ in range(n):
                            nc.tensor.matmul(
                                ps_warm, wtile[:, 0:C], wtile,
                                start=True, stop=True,
                            )

                    def warm_tail(n):
                        # warms pinned on the LAST x chunk: fill the PE gap
                        # between the pool matmuls and the conv
                        for _ in range(n):
                            nc.tensor.matmul(
                                ps_warm, wtile[:, 0:C],
                                xs[:, ds(H * W - 512, 512)],
                                start=True, stop=True,
                            )

                    # ---- short PE ramp before chunk 0 lands
                    warm(2)

                    # ---- chunk-chasing ----
                    for r0, nr in XCH:
                        for g4 in range(r0, r0 + nr, 4):
                            nc.tensor.matmul(
                                ps_xw,
                                w1ts_sb[:, min(g4 // 32, 2), :],
                                xs[:, ds(g4 * W, 4 * W)],
                                start=(g4 == 0),
                                stop=(g4 == H - 4),
                                skip_group_check=True,
                            )
                        if r0 < 64:
                            # x_h seg-matmuls, 16-row psum slices
                            for b0 in range(r0, r0 + nr, 16):
                                nb = min(16, r0 + nr - b0)
                                xv = xs[:, ds(b0 * W, nb * W)].rearrange(
                                    "p (y q s) -> p y q s", q=8, s=16
                                )
                                for s in range(8):
                                    nc.tensor.matmul(
                                        ps_xh0[:, ds(b0, nb), :],
                                        w1ts_sb[:, min(s // 2, 2), :],
                                        xv[:, :, s, :],
                                        start=(s == 0),
                                        stop=(s == 7),
                                        skip_group_check=True,
                                    )
                        if r0 + nr == 64:
                            # rows 0-63 x_h: reduce psum -> yh directly (w1
                            # already applied), then bn + a_h matmul+sigmoid
                            yh0 = small.tile([MIP, 64], F32)
                            nc.vector.tensor_reduce(
                                out=yh0,
                                in_=ps_xh0,
                                axis=mybir.AxisListType.X,
                                op=ALU.add,
                            )
                            xhsA = pp.tile([MIP, 64], BF16, tag="xh_sh0")
                            bn_hswish(yh0, xhsA, 64)
                            nc.tensor.matmul(
                                ps_ah[:, 0:64], wht_sb, xhsA,
                                start=True, stop=True,
                            )
                            nc.scalar.activation(
                                ah_sb[:, 0:64], ps_ah[:, 0:64],
                                AF.Sigmoid, bias=p128_sb[:, 0:1], scale=1.0,
                            )

                    # ---- a_w chain: the critical path ----
                    # All-vector: trees 80-127 are deferred behind the first
                    # gates, so the vector queue is clear the moment the stop
                    # matmul lands. (Pool is NOT an option: its software ops
                    # cost ~2us for scalar-ptr / max-min, and it cannot read
                    # PSUM.)
                    xwp = small.tile([MIP, W], F32)
                    nc.vector.tensor_reduce(
                        out=xwp,
                        in_=ps_xw.rearrange("p j x -> p x j"),
                        axis=mybir.AxisListType.X,
                        op=ALU.add,
                    )
                    xw_s = small.tile([MIP, W], BF16)
                    bn_hswish(xwp, xw_s, W)
                    nc.tensor.matmul(ps_aw, wwt_sb, xw_s, start=True, stop=True)
                    aw_sb = small.tile([C, W], BF16)
                    nc.scalar.activation(
                        aw_sb, ps_aw, AF.Sigmoid, bias=p128_sb[:, 1:2], scale=1.0
                    )

                    # gates 0-63 unblock the conv ASAP; everything for rows
                    # 64-127 queues behind them (conv reaches row 64 only
                    # ~30us after it starts)
                    gate_rows(0, 64)

                    # rows 64-127 x_h path (deferred trees + combine): pooled
                    # -> yh matmul -> bn -> a_h matmul+tanh. Fully off the
                    # critical path (conv reaches row 64 ~35us later).
                    # xhpB uses integer weights 7/3/1; the stationary
                    # w1ts[2] = w1/128 then yields exactly
                    # w1 @ (7*S0 + 3*S1 + S2 + S3)/128 = w1 @ x_h-pool
                    emit_tree(64, 16)
                    emit_tree(80, 16)
                    emit_tree(96, 16)
                    emit_tree(112, 16)
                    tmpA = pp.tile([C, 64], F32, tag="tmpA")
                    nc.vector.tensor_add(tmpA, s32b[:, :, 2], s32b[:, :, 3])
                    m0 = pp.tile([C, 64], F32, tag="m0")
                    nc.vector.tensor_scalar_mul(m0, s32b[:, :, 0], 7.0)
                    m1 = pp.tile([C, 64], F32, tag="m1")
                    nc.vector.scalar_tensor_tensor(
                        out=m1, in0=s32b[:, :, 1], scalar=3.0, in1=m0,
                        op0=ALU.mult, op1=ALU.add,
                    )
                    xhpB = pp.tile([C, 64], BF16, tag="xhpB")
                    nc.vector.tensor_add(xhpB, tmpA, m1)

                    # bridge the post-pool PE idle until the conv starts;
                    # sized so the tails run dry just before the first conv
                    # matmul becomes ready (leftover tails would delay it)
                    warm_tail(14)

                # ---- 3x3 conv + BN2 + SiLU ----
                # The rows-64-127 a_h PE matmuls are spliced in after conv
                # block 10: their vector feeders complete behind gates 0-63,
                # and a stalled LDWEIGHTS earlier in the PE queue blocks all
                # later matmuls (no bypass once a stationary starts loading).
                # By block 11 the feeders are long ready, and gates 64-127
                # still land ~10us before conv block 16 consumes them.
                with (
                    tc.tile_pool(name="psB", bufs=5, space="PSUM") as psB,
                    tc.tile_pool(name="obp", bufs=4) as obp,
                ):
                    def conv_block(rb):
                        pso = psB.tile([C, 4, W], F32, tag="pso")
                        for k in range(9):
                            dy, dx = k // 3, k % 3
                            nc.tensor.matmul(
                                pso,
                                wct_sb[:, k, :],
                                ug[:, 4 * rb + dy : 4 * rb + dy + 4,
                                   1 + dx : 1 + dx + W],
                                start=(k == 0),
                                stop=(k == 8),
                            )
                        ob = obp.tile([C, 4, W], BF16, tag="ob")
                        nc.scalar.activation(
                            ob, pso, AF.Silu,
                            bias=p128_sb[:, 3:4], scale=p128_sb[:, 2:3],
                        )
                        nc.sync.dma_start(
                            out=outp[:, 4 * rb : 4 * rb + 4, :], in_=ob
                        )

                    for rb in range(12):
                        conv_block(rb)

                    nc.tensor.matmul(
                        ps_yh, w1ts_sb[:, 2, :], xhpB, start=True, stop=True
                    )
                    xhsB = pp.tile([MIP, 64], BF16, tag="xh_sh64")
                    bn_hswish(ps_yh, xhsB, 64)
                    ah_mm_tanh(64, 128, xhsB)
                    gate_rows(64, 128)

                    for rb in range(12, H // 4):
                        conv_block(rb)

    nc.compile()
    return nc


def prep_inputs(x, w1, b1, g1, be1, m1, v1, wh, bh, ww, bw, wc, bc, g2, be2, m2, v2):
    """Host-side prep: per-core input maps (weights replicated)."""
    bf = ml_dtypes.bfloat16
    N = x.shape[0]
    s1 = (g1 / np.sqrt(v1 + EPS)).astype(np.float64)
    t1f = s1 * b1 + be1 - m1 * s1
    # bn_hswish consts: v = (s1/6)*x + (t1f/6 + 0.5); cols 2-3 unused
    p8 = np.stack(
        [s1 / 6.0, t1f / 6.0 + 0.5, 0.0 * s1, 0.0 * s1], axis=1
    ).astype(np.float32)
    s2 = (g2 / np.sqrt(v2 + EPS)).astype(np.float64)
    b2 = bc * s2 + be2 - m2 * s2
    # pcomb cols: 0 bh, 1 bw (sigmoid path), 2 s2, 3 b2, 4 bh/2 (tanh
    # path); cols 6-9 rows 0-7: the p8 block
    pcomb = np.zeros((C, 10), np.float32)
    pcomb[:, 0:4] = np.stack([bh, bw, s2, b2], axis=1)
    pcomb[:, 4] = 0.5 * bh
    pcomb[0:MIP, 6:10] = p8
    cc = np.array([7.0 / 128, 3.0 / 128, 1.0 / 128])
    w1ts = np.stack([w1.T * c for c in cc], axis=1)              # (C, 3, MIP)
    shared = {
        "w1ts": np.ascontiguousarray(w1ts.reshape(C, 3 * MIP)).astype(bf),
        "whw": np.ascontiguousarray(
            6.0 * np.concatenate([wh.T, ww.T], axis=1)
        ).astype(bf),                                            # (MIP, 2C)
        "wct": np.ascontiguousarray(
            np.transpose(wc, (1, 2, 3, 0)).reshape(C, 9 * C)
        ).astype(bf),                                            # [i, (ky kx), o]
        "pcomb": pcomb,
    }
    in_maps = []
    for n in range(N):
        m = dict(shared)
        m["x"] = np.ascontiguousarray(x[n].reshape(C, H * W)).astype(bf)
        in_maps.append(m)
    return in_maps


def run(inputs, trace=False):
    if "nc" not in _CACHE:
        _CACHE["nc"] = build_nc()
    nc = _CACHE["nc"]
    in_maps = prep_inputs(**inputs)
    res = run_bass_kernel_spmd(nc, in_maps, core_ids=list(range(8)), trace=trace)
    out = np.stack([np.asarray(res.results[i]["out"]) for i in range(8)], axis=0)
    return out.astype(np.float32), res


def kernel(**inputs) -> np.ndarray:
    out, _ = run(inputs, trace=False)
    return out


# revision 50
# speedup vs baseline: 1.1825x; 1.0036x over previous
"""Trainium2 Bass kernel for CAConv2 (coordinate-attention + 3x3 conv block).

Shapes (hardcoded): x (8, 128, 128, 128) f32; data-parallel over batch,
one image per NeuronCore (8 cores).

Scheduling facts learned from HW traces / the cost model:
- PE streams conv matmuls at ~216ns/512 cols (~2.3 GHz effective) once
  warm; the 3x3 conv (32 blocks x 9 matmuls) is a ~63us floor and
  everything else must hide around it.
- Engines run their queues in order with a ~4-deep stalled-instruction
  bypass window: emission position is priority, dependencies are
  eligibility. A stalled matmul whose LDWEIGHTS has issued blocks ALL
  later matmuls, so PE matmuls with late feeders must be emitted at a
  queue position where their inputs are certainly ready (the B-path
  a_h matmuls are spliced between conv blocks 11 and 12).
- Pool (gpsimd) elementwise ops halve concurrent PE/DVE throughput
  (SBUF contention) and cannot touch PSUM; scalar-ptr / max-min Pool
  ops cost ~2us. Pool is left idle on purpose.
- DMA engines (16) sustain ~21 GB/s each (~300-340 GB/s aggregate); x
  (4MB bf16) needs ~13.5us of drain. All loads ride the sync HW ring
  (the gpsimd SW ring has ~1us wakeup and its completion semaphores
  stall behind trigger issuance); wct queues behind all of x on the
  same in-order ring. DMA triggers cost ~0.6-0.8us each on the issuing
  queue and completion semaphores fire ~1-2us after last data (engine
  stagger).
- ACT tables load per *set*: sigmoid set first for the critical a_w /
  a_h(0-63) activations; the single switch to the silu set lands in the
  ACT-idle window before the first conv Silu; the mid-conv a_h(64-127)
  sigmoid is computed as 0.5+0.5*tanh(z/2) (Tanh is in the silu set).
- Cross-engine dependency hops cost ~0.3-1us; run-to-run variance is
  ~+-1.5us (DMA ring behavior), and the device itself oscillates between
  a ~2.35GHz and a ~2.0GHz PE regime on minute timescales (conv cadence
  1965 vs 2377 ns/block) -- never judge a change on one run.
- The a_w critical chain is latency-tuned: 3-op bn+hswish (the /6 folded
  into 6x-scaled wh/ww weights), and the gating tg tiles live in a
  bufs=1 pool so at most ONE early-ready pass1 op can jump into the bn
  chain's semaphore gaps (awred->sigmoid = 2.64us, deterministic).
"""

import numpy as np
import ml_dtypes

import concourse.bacc as bacc
import concourse.tile as tile
from concourse import mybir
from concourse.bass import ds
from concourse.bass_utils import run_bass_kernel_spmd

BF16 = mybir.dt.bfloat16
F32 = mybir.dt.float32
C, H, W, MIP = 128, 128, 128, 8
WP = W + 4  # padded width: cols [2, 130) hold data, 0/1 and 130/131 are zero
HP = H + 2  # padded height: rows [1, 129) hold data
EPS = 1e-5
AF = mybir.ActivationFunctionType
ALU = mybir.AluOpType

_CACHE = {}


def build_nc():
    nc = bacc.Bacc(num_swdge_queues=1)
    xp = nc.declare_dram_parameter("x", [C, H * W], BF16, isOutput=False)
    w1ts = nc.declare_dram_parameter("w1ts", [C, 3 * MIP], BF16, isOutput=False)
    # wht | wwt packed side by side (one DMA trigger)
    whw = nc.declare_dram_parameter("whw", [MIP, 2 * C], BF16, isOutput=False)
    # wct[i, k, o] = wc[o, i, k//3, k%3]
    wct = nc.declare_dram_parameter("wct", [C, 9 * C], BF16, isOutput=False)
    # pcomb cols (all 128 rows): 0 bh, 1 bw, 2 s2, 3 b2 (= bc*s2+be2-m2*s2),
    # 4 bh/2 (tanh path); cols 6-9 (rows 0-7): s1/6, t1f/6, s1, t1f+3
    pcomb = nc.declare_dram_parameter("pcomb", [C, 10], F32, isOutput=False)
    outp = nc.declare_dram_parameter("out", [C, H, W], BF16, isOutput=True)

    with tile.TileContext(nc) as tc:
        with (
            tc.tile_pool(name="sing", bufs=1) as sing,
            tc.tile_pool(name="pp", bufs=2) as pp,
            # tg gets bufs=1 on purpose: DVE is serial so strict
            # pass1/pass2 alternation costs nothing, but it caps the
            # early-ready pass1 ops that can jump into the a_w bn chain's
            # semaphore gaps at ONE (each injection costs ~0.6us of
            # critical path)
            tc.tile_pool(name="tgp", bufs=1) as tgp,
            tc.tile_pool(name="small", bufs=1) as small,
        ):
            xs = sing.tile([C, H * W], BF16)
            ug = sing.tile([C, HP, WP], BF16)
            s32b = sing.tile([C, 64, 4], F32)  # col-segment sums rows 64-127
            wtile = sing.tile([C, 512], BF16)  # zeros; warm-matmul fodder

            # Everything rides the sync HW ring: the gpsimd SWDGE ring has a
            # ~0.9us wakeup latency and its completion semaphores stall
            # behind trigger issuance on the GpSimd sequencer. The HW ring is
            # in-order per engine, so interleaving small weight loads between
            # x chunks preserves the chunk-chase sequencing, and wct's 288KB
            # queue behind all of x (no HBM contention mid-stream).
            XCH = [(0, 16), (16, 16), (32, 32), (64, 32), (96, 16), (112, 8), (120, 8)]
            w1ts_sb = sing.tile([C, 3, MIP], BF16)
            whw_sb = sing.tile([MIP, 2 * C], BF16)
            pc_sb = sing.tile([C, 10], F32)
            wct_sb = sing.tile([C, 9, C], BF16)
            side = {
                0: [(w1ts_sb, w1ts.rearrange("c (r m) -> c r m", r=3))],
            }
            # (Tried: last x chunks on the gpsimd SW ring to dodge the tail
            # stagger — regressed ~10us; bulk transfers on the two rings do
            # not overlap cleanly. Tiny side loads are fine there though.)
            for ci, (r0, nr) in enumerate(XCH):
                nc.sync.dma_start(
                    out=xs[:, ds(r0 * W, nr * W)],
                    in_=xp[:, ds(r0 * W, nr * W)],
                )
                for dst, src in side.get(ci, []):
                    nc.sync.dma_start(out=dst, in_=src)
            nc.sync.dma_start(out=wct_sb, in_=wct.rearrange("i (k o) -> i k o", k=9))
            # pcomb/whw ride the gpsimd ring: ~9KB total, needed only at
            # ~+14us, and keeping the sync ring at 9 triggers stays within
            # the DMA semaphore pool (no reuse-wait stalls on x triggers).
            nc.gpsimd.dma_start(out=pc_sb, in_=pcomb[:, :])
            nc.gpsimd.dma_start(out=whw_sb, in_=whw[:, :])
            wht_sb = whw_sb[:, 0:C]
            wwt_sb = whw_sb[:, C : 2 * C]
            p128_sb = pc_sb[:, 0:5]
            p8_sb = pc_sb[0:MIP, 6:10]

            # warm fodder first so the PE can start ramping ASAP
            nc.vector.memset(wtile, 0.0)
            # conv padding border of ug
            nc.vector.memset(ug[:, 0, :], 0.0)
            nc.vector.memset(ug[:, HP - 1, :], 0.0)
            nc.vector.memset(ug[:, 1 : HP - 1, 0:2], 0.0)
            nc.vector.memset(ug[:, 1 : HP - 1, WP - 2 : WP], 0.0)

            # ACT tables: preload the sigmoid set for the a_w / a_h(0-63)
            # activations (critical path); the compiler inserts ONE switch to
            # the silu set after the last sigmoid, landing in the ACT-idle
            # window before the first conv Silu. The mid-conv a_h(64-127)
            # activation uses Tanh, which the silu set also contains.
            dump = small.tile([C, 2], F32)
            nc.scalar.activation(dump, wtile[:, 0:2], AF.Sigmoid, bias=0.0, scale=1.0)

            with tc.tile_pool(name="psA", bufs=1, space="PSUM") as psA:
                ps_yh = psA.tile([MIP, 64], F32, tag="yh")
                ps_ah = psA.tile([C, H], F32, tag="ah")
                ps_aw = psA.tile([C, W], F32, tag="aw")
                ah_sb = small.tile([C, H], BF16)
                ah_t = small.tile([C, H], F32)  # raw tanh before affine

                def emit_tree(r0, nr):
                    # 32-col segment sums for rows [r0, r0+nr) (rows >= 64).
                    # DVE only: Pool elementwise ops halve concurrent PE/DVE
                    # throughput (SBUF contention), so Pool stays idle.
                    eng = nc.vector
                    xc = xs[:, ds(r0 * W, nr * W)].rearrange(
                        "p (y q s) -> p y q s", q=4, s=32
                    )
                    t1 = pp.tile([C, 16, 4, 16], BF16, tag="t1")
                    eng.tensor_add(t1[:, :nr], xc[:, :, :, 0:16], xc[:, :, :, 16:32])
                    t2 = pp.tile([C, 16, 4, 8], BF16, tag="t2")
                    eng.tensor_add(t2[:, :nr], t1[:, :nr, :, 0:8], t1[:, :nr, :, 8:16])
                    t3 = pp.tile([C, 16, 4, 4], BF16, tag="t3")
                    eng.tensor_add(t3[:, :nr], t2[:, :nr, :, 0:4], t2[:, :nr, :, 4:8])
                    t4 = pp.tile([C, 16, 4, 2], BF16, tag="t4")
                    eng.tensor_add(t4[:, :nr], t3[:, :nr, :, 0:2], t3[:, :nr, :, 2:4])
                    eng.tensor_add(
                        s32b[:, ds(r0 - 64, nr), :], t4[:, :nr, :, 0], t4[:, :nr, :, 1]
                    )

                def bn_hswish(src, dst, n, eng=None):
                    # dst = h_swish(s1*src + t1f) / 6 in THREE ops (the /6 is
                    # repaid by scaling the downstream wh/ww weights by 6):
                    # v = (u+3)/6, w = clamp(v,0,1), dst = (v-0.5)*w
                    eng = eng or nc.vector
                    v = pp.tile([MIP, n], F32, tag="bn_v")
                    eng.tensor_scalar(
                        out=v, in0=src, scalar1=p8_sb[:, 0:1],
                        scalar2=p8_sb[:, 1:2], op0=ALU.mult, op1=ALU.add,
                    )
                    w = pp.tile([MIP, n], F32, tag="bn_w")
                    eng.tensor_scalar(
                        out=w, in0=v, scalar1=0.0, scalar2=1.0,
                        op0=ALU.max, op1=ALU.min,
                    )
                    eng.scalar_tensor_tensor(
                        out=dst, in0=v, scalar=0.5, in1=w,
                        op0=ALU.subtract, op1=ALU.mult,
                    )

                def ah_mm_tanh(rlo, rhi, xh_sh):
                    # a_h = sigmoid(wh @ xh + bh) via 0.5+0.5*tanh(.5z+.5bh):
                    # runs mid-conv when the silu table is resident (the silu
                    # set contains Tanh, so no table switch is triggered)
                    nc.tensor.matmul(
                        ps_ah[:, ds(rlo, rhi - rlo)], wht_sb, xh_sh,
                        start=True, stop=True,
                    )
                    nc.scalar.activation(
                        ah_t[:, ds(rlo, rhi - rlo)], ps_ah[:, ds(rlo, rhi - rlo)],
                        AF.Tanh, bias=p128_sb[:, 4:5], scale=0.5,
                    )
                    nc.vector.tensor_scalar(
                        out=ah_sb[:, ds(rlo, rhi - rlo)],
                        in0=ah_t[:, ds(rlo, rhi - rlo)],
                        scalar1=0.5, scalar2=0.5, op0=ALU.mult, op1=ALU.add,
                    )

                def gate_rows(rlo, rhi):
                    # ug rows = x * a_h[c,y] * a_w[c,x], 4 rows per op pair:
                    # pass1 multiplies by a_h (stride-0 bcast along x, 1x DVE
                    # rate) — it only needs a_h, so the vector queue's bypass
                    # window precomputes the first tg tiles while the a_w
                    # chain is still running; pass2 (packed, 2x rate) applies
                    # a_w. ~30% cheaper than per-row scalar_tensor_tensor.
                    aw_b = aw_sb[:, None, :].broadcast_to([C, 4, W])
                    for y in range(rlo, rhi, 4):
                        tg = tgp.tile([C, 4, W], BF16, tag="tg")
                        x4 = xs[:, ds(y * W, 4 * W)].rearrange(
                            "p (r x) -> p r x", r=4
                        )
                        ah_b = ah_sb[:, y : y + 4, None].broadcast_to([C, 4, W])
                        nc.vector.tensor_mul(tg, x4, ah_b)
                        nc.vector.tensor_mul(
                            ug[:, 1 + y : 5 + y, 2 : 2 + W], tg, aw_b
                        )

                with tc.tile_pool(name="psW", bufs=1, space="PSUM") as psW:
                    ps_warm = psW.tile([C, 512], F32, tag="warm")
                    # x_w row-pool: 4-row matmuls with range-prescaled w1
                    # accumulate onto ONE (8, 4, W) psum bank
                    ps_xw = psW.tile([MIP, 4, W], F32, tag="xw")
                    # x_h col-pool for rows 0-63: per 16-row group, 8
                    # 16-col seg-matmuls (weights c1/c1/c2/c2/c3.. per seg)
                    # accumulate w1-projected partial sums
                    ps_xh0 = psW.tile([MIP, 64, 16], F32, tag="xh0")

                    def warm(n):
                        # dep-free warms: bridge PE gaps / hold the p-state
                        for _ in range(n):
                            nc.tensor.matmul(
                                ps_warm, wtile[:, 0:C], wtile,
                                start=True, stop=True,
                            )

                    def warm_tail(n):
                        # warms pinned on the LAST x chunk: fill the PE gap
                        # between the pool matmuls and the conv
                        for _ in range(n):
                            nc.tensor.matmul(
                                ps_warm, wtile[:, 0:C],
                                xs[:, ds(H * W - 512, 512)],
                                start=True, stop=True,
                            )

                    # ---- short PE ramp before chunk 0 lands
                    warm(2)

                    # ---- chunk-chasing ----
                    for r0, nr in XCH:
                        for g4 in range(r0, r0 + nr, 4):
                            nc.tensor.matmul(
                                ps_xw,
                                w1ts_sb[:, min(g4 // 32, 2), :],
                                xs[:, ds(g4 * W, 4 * W)],
                                start=(g4 == 0),
                                stop=(g4 == H - 4),
                                skip_group_check=True,
                            )
                        if r0 < 64:
                            # x_h seg-matmuls, 16-row psum slices
                            for b0 in range(r0, r0 + nr, 16):
                                nb = min(16, r0 + nr - b0)
                                xv = xs[:, ds(b0 * W, nb * W)].rearrange(
                                    "p (y q s) -> p y q s", q=8, s=16
                                )
                                for s in range(8):
                                    nc.tensor.matmul(
                                        ps_xh0[:, ds(b0, nb), :],
                                        w1ts_sb[:, min(s // 2, 2), :],
                                        xv[:, :, s, :],
                                        start=(s == 0),
                                        stop=(s == 7),
                                        skip_group_check=True,
                                    )
                        if r0 + nr == 64:
                            # rows 0-63 x_h: reduce psum -> yh directly (w1
                            # already applied), then bn + a_h matmul+sigmoid
                            yh0 = small.tile([MIP, 64], F32)
                            nc.vector.tensor_reduce(
                                out=yh0,
                                in_=ps_xh0,
                                axis=mybir.AxisListType.X,
                                op=ALU.add,
                            )
                            xhsA = pp.tile([MIP, 64], BF16, tag="xh_sh0")
                            bn_hswish(yh0, xhsA, 64)
                            nc.tensor.matmul(
                                ps_ah[:, 0:64], wht_sb, xhsA,
                                start=True, stop=True,
                            )
                            nc.scalar.activation(
                                ah_sb[:, 0:64], ps_ah[:, 0:64],
                                AF.Sigmoid, bias=p128_sb[:, 0:1], scale=1.0,
                            )

                    # ---- a_w chain: the critical path ----
                    # All-vector: trees 80-127 are deferred behind the first
                    # gates, so the vector queue is clear the moment the stop
                    # matmul lands. (Pool is NOT an option: its software ops
                    # cost ~2us for scalar-ptr / max-min, and it cannot read
                    # PSUM.)
                    xwp = small.tile([MIP, W], F32)
                    nc.vector.tensor_reduce(
                        out=xwp,
                        in_=ps_xw.rearrange("p j x -> p x j"),
                        axis=mybir.AxisListType.X,
                        op=ALU.add,
                    )
                    xw_s = small.tile([MIP, W], BF16)
                    bn_hswish(xwp, xw_s, W)
                    nc.tensor.matmul(ps_aw, wwt_sb, xw_s, start=True, stop=True)
                    aw_sb = small.tile([C, W], BF16)
                    nc.scalar.activation(
                        aw_sb, ps_aw, AF.Sigmoid, bias=p128_sb[:, 1:2], scale=1.0
                    )

                    # gates 0-63 unblock the conv ASAP; everything for rows
                    # 64-127 queues behind them (conv reaches row 64 only
                    # ~30us after it starts)
                    gate_rows(0, 64)

                    # rows 64-127 x_h path (deferred trees + combine): pooled
                    # -> yh matmul -> bn -> a_h matmul+tanh. Fully off the
                    # critical path (conv reaches row 64 ~35us later).
                    # xhpB uses integer weights 7/3/1; the stationary
                    # w1ts[2] = w1/128 then yields exactly
                    # w1 @ (7*S0 + 3*S1 + S2 + S3)/128 = w1 @ x_h-pool
                    emit_tree(64, 16)
                    emit_tree(80, 16)
                    emit_tree(96, 16)
                    emit_tree(112, 16)
                    tmpA = pp.tile([C, 64], F32, tag="tmpA")
                    nc.vector.tensor_add(tmpA, s32b[:, :, 2], s32b[:, :, 3])
                    m0 = pp.tile([C, 64], F32, tag="m0")
                    nc.vector.tensor_scalar_mul(m0, s32b[:, :, 0], 7.0)
                    m1 = pp.tile([C, 64], F32, tag="m1")
                    nc.vector.scalar_tensor_tensor(
                        out=m1, in0=s32b[:, :, 1], scalar=3.0, in1=m0,
                        op0=ALU.mult, op1=ALU.add,
                    )
                    xhpB = pp.tile([C, 64], BF16, tag="xhpB")
                    nc.vector.tensor_add(xhpB, tmpA, m1)

                    # bridge the post-pool PE idle until the conv starts;
                    # sized so the tails run dry just before the first conv
                    # matmul becomes ready (leftover tails would delay it)
                    warm_tail(14)

                # ---- 3x3 conv + BN2 + SiLU ----
                # The rows-64-127 a_h PE matmuls are spliced in after conv
                # block 10: their vector feeders complete behind gates 0-63,
                # and a stalled LDWEIGHTS earlier in the PE queue blocks all
                # later matmuls (no bypass once a stationary starts loading).
                # By block 11 the feeders are long ready, and gates 64-127
                # still land ~10us before conv block 16 consumes them.
                with (
                    tc.tile_pool(name="psB", bufs=5, space="PSUM") as psB,
                    tc.tile_pool(name="obp", bufs=4) as obp,
                ):
                    def conv_block(rb):
                        pso = psB.tile([C, 4, W], F32, tag="pso")
                        for k in range(9):
                            dy, dx = k // 3, k % 3
                            nc.tensor.matmul(
                                pso,
                                wct_sb[:, k, :],
                                ug[:, 4 * rb + dy : 4 * rb + dy + 4,
                                   1 + dx : 1 + dx + W],
                                start=(k == 0),
                                stop=(k == 8),
                            )
                        ob = obp.tile([C, 4, W], BF16, tag="ob")
                        nc.scalar.activation(
                            ob, pso, AF.Silu,
                            bias=p128_sb[:, 3:4], scale=p128_sb[:, 2:3],
                        )
                        nc.sync.dma_start(
                            out=outp[:, 4 * rb : 4 * rb + 4, :], in_=ob
                        )

                    for rb in range(12):
                        conv_block(rb)

                    nc.tensor.matmul(
                        ps_yh, w1ts_sb[:, 2, :], xhpB, start=True, stop=True
                    )
                    xhsB = pp.tile([MIP, 64], BF16, tag="xh_sh64")
                    bn_hswish(ps_yh, xhsB, 64)
                    ah_mm_tanh(64, 128, xhsB)
                    gate_rows(64, 128)

                    for rb in range(12, H // 4):
                        conv_block(rb)

    nc.compile()
    return nc


def prep_inputs(x, w1, b1, g1, be1, m1, v1, wh, bh, ww, bw, wc, bc, g2, be2, m2, v2):
    """Host-side prep: per-core input maps (weights replicated)."""
    bf = ml_dtypes.bfloat16
    N = x.shape[0]
    s1 = (g1 / np.sqrt(v1 + EPS)).astype(np.float64)
    t1f = s1 * b1 + be1 - m1 * s1
    # bn_hswish consts: v = (s1/6)*x + (t1f/6 + 0.5); cols 2-3 unused
    p8 = np.stack(
        [s1 / 6.0, t1f / 6.0 + 0.5, 0.0 * s1, 0.0 * s1], axis=1
    ).astype(np.float32)
    s2 = (g2 / np.sqrt(v2 + EPS)).astype(np.float64)
    b2 = bc * s2 + be2 - m2 * s2
    # pcomb cols: 0 bh, 1 bw (sigmoid path), 2 s2, 3 b2, 4 bh/2 (tanh
    # path); cols 6-9 rows 0-7: the p8 block
    pcomb = np.zeros((C, 10), np.float32)
    pcomb[:, 0:4] = np.stack([bh, bw, s2, b2], axis=1)
    pcomb[:, 4] = 0.5 * bh
    pcomb[0:MIP, 6:10] = p8
    cc = np.array([7.0 / 128, 3.0 / 128, 1.0 / 128])
    w1ts = np.stack([w1.T * c for c in cc], axis=1)              # (C, 3, MIP)
    shared = {
        "w1ts": np.ascontiguousarray(w1ts.reshape(C, 3 * MIP)).astype(bf),
        "whw": np.ascontiguousarray(
            6.0 * np.concatenate([wh.T, ww.T], axis=1)
        ).astype(bf),                                            # (MIP, 2C)
        "wct": np.ascontiguousarray(
            np.transpose(wc, (1, 2, 3, 0)).reshape(C, 9 * C)
        ).astype(bf),                                            # [i, (ky kx), o]
        "pcomb": pcomb,
    }
    in_maps = []
    for n in range(N):
        m = dict(shared)
        m["x"] = np.ascontiguousarray(x[n].reshape(C, H * W)).astype(bf)
        in_maps.append(m)
    return in_maps


def run(inputs, trace=False):
    if "nc" not in _CACHE:
        _CACHE["nc"] = build_nc()
    nc = _CACHE["nc"]
    in_maps = prep_inputs(**inputs)
    res = run_bass_kernel_spmd(nc, in_maps, core_ids=list(range(8)), trace=trace)
    out = np.stack([np.asarray(res.results[i]["out"]) for i in range(8)], axis=0)
    return out.astype(np.float32), res


def kernel(**inputs) -> np.ndarray:
    out, _ = run(inputs, trace=False)
    return out


# revision 51
# speedup vs baseline: 1.1853x; 1.0024x over previous
"""Trainium2 Bass kernel for CAConv2 (coordinate-attention + 3x3 conv block).

Shapes (hardcoded): x (8, 128, 128, 128) f32; data-parallel over batch,
one image per NeuronCore (8 cores).

Scheduling facts learned from HW traces / the cost model:
- PE streams conv matmuls at ~216ns/512 cols (~2.3 GHz effective) once
  warm; the 3x3 conv (32 blocks x 9 matmuls) is a ~63us floor and
  everything else must hide around it.
- Engines run their queues in order with a ~4-deep stalled-instruction
  bypass window: emission position is priority, dependencies are
  eligibility. A stalled matmul whose LDWEIGHTS has issued blocks ALL
  later matmuls, so PE matmuls with late feeders must be emitted at a
  queue position where their inputs are certainly ready (the B-path
  a_h matmuls are spliced between conv blocks 11 and 12).
- Pool (gpsimd) elementwise ops halve concurrent PE/DVE throughput
  (SBUF contention) and cannot touch PSUM; scalar-ptr / max-min Pool
  ops cost ~2us. Pool is left idle on purpose.
- DMA engines (16) sustain ~21 GB/s each (~300-340 GB/s aggregate); x
  (4MB bf16) needs ~13.5us of drain. All loads ride the sync HW ring
  (the gpsimd SW ring has ~1us wakeup and its completion semaphores
  stall behind trigger issuance); wct queues behind all of x on the
  same in-order ring. DMA triggers cost ~0.6-0.8us each on the issuing
  queue and completion semaphores fire ~1-2us after last data (engine
  stagger).
- ACT tables load per *set*: sigmoid set first for the critical a_w /
  a_h(0-63) activations; the single switch to the silu set lands in the
  ACT-idle window before the first conv Silu; the mid-conv a_h(64-127)
  sigmoid is computed as 0.5+0.5*tanh(z/2) (Tanh is in the silu set).
- Cross-engine dependency hops cost ~0.3-1us; run-to-run variance is
  ~+-1.5us (DMA ring behavior), and the device itself oscillates between
  a ~2.35GHz and a ~2.0GHz PE regime on minute timescales (conv cadence
  1965 vs 2377 ns/block) -- never judge a change on one run.
- The a_w critical chain is latency-tuned: 3-op bn+hswish (the /6 folded
  into 6x-scaled wh/ww weights), and the gating tg tiles live in a
  bufs=1 pool so at most ONE early-ready pass1 op can jump into the bn
  chain's semaphore gaps (awred->sigmoid = 2.64us, deterministic).
"""

import numpy as np
import ml_dtypes

import concourse.bacc as bacc
import concourse.tile as tile
from concourse import mybir
from concourse.bass import ds
from concourse.bass_utils import run_bass_kernel_spmd

BF16 = mybir.dt.bfloat16
F32 = mybir.dt.float32
C, H, W, MIP = 128, 128, 128, 8
WP = W + 4  # padded width: cols [2, 130) hold data, 0/1 and 130/131 are zero
HP = H + 2  # padded height: rows [1, 129) hold data
EPS = 1e-5
AF = mybir.ActivationFunctionType
ALU = mybir.AluOpType

_CACHE = {}


def build_nc():
    nc = bacc.Bacc(num_swdge_queues=1)
    xp = nc.declare_dram_parameter("x", [C, H * W], BF16, isOutput=False)
    w1ts = nc.declare_dram_parameter("w1ts", [C, 3 * MIP], BF16, isOutput=False)
    # wht | wwt packed side by side (one DMA trigger)
    whw = nc.declare_dram_parameter("whw", [MIP, 2 * C], BF16, isOutput=False)
    # wct[i, k, o] = wc[o, i, k//3, k%3]
    wct = nc.declare_dram_parameter("wct", [C, 9 * C], BF16, isOutput=False)
    # pcomb cols (all 128 rows): 0 bh, 1 bw, 2 s2, 3 b2 (= bc*s2+be2-m2*s2),
    # 4 bh/2 (tanh path); cols 6-9 (rows 0-7): s1/6, t1f/6, s1, t1f+3
    pcomb = nc.declare_dram_parameter("pcomb", [C, 10], F32, isOutput=False)
    outp = nc.declare_dram_parameter("out", [C, H, W], BF16, isOutput=True)

    with tile.TileContext(nc) as tc:
        with (
            tc.tile_pool(name="sing", bufs=1) as sing,
            tc.tile_pool(name="pp", bufs=2) as pp,
            # tg gets bufs=1 on purpose: DVE is serial so strict
            # pass1/pass2 alternation costs nothing, but it caps the
            # early-ready pass1 ops that can jump into the a_w bn chain's
            # semaphore gaps at ONE (each injection costs ~0.6us of
            # critical path)
            tc.tile_pool(name="tgp", bufs=1) as tgp,
            tc.tile_pool(name="small", bufs=1) as small,
        ):
            xs = sing.tile([C, H * W], BF16)
            ug = sing.tile([C, HP, WP], BF16)
            s32b = sing.tile([C, 64, 4], F32)  # col-segment sums rows 64-127
            wtile = sing.tile([C, 512], BF16)  # zeros; warm-matmul fodder

            # Everything rides the sync HW ring: the gpsimd SWDGE ring has a
            # ~0.9us wakeup latency and its completion semaphores stall
            # behind trigger issuance on the GpSimd sequencer. The HW ring is
            # in-order per engine, so interleaving small weight loads between
            # x chunks preserves the chunk-chase sequencing, and wct's 288KB
            # queue behind all of x (no HBM contention mid-stream).
            XCH = [(0, 16), (16, 16), (32, 32), (64, 32), (96, 16), (112, 8), (120, 8)]
            w1ts_sb = sing.tile([C, 3, MIP], BF16)
            whw_sb = sing.tile([MIP, 2 * C], BF16)
            pc_sb = sing.tile([C, 10], F32)
            wct_sb = sing.tile([C, 9, C], BF16)
            side = {
                0: [(w1ts_sb, w1ts.rearrange("c (r m) -> c r m", r=3))],
            }
            # (Tried: last x chunks on the gpsimd SW ring to dodge the tail
            # stagger — regressed ~10us; bulk transfers on the two rings do
            # not overlap cleanly. Tiny side loads are fine there though.)
            for ci, (r0, nr) in enumerate(XCH):
                nc.sync.dma_start(
                    out=xs[:, ds(r0 * W, nr * W)],
                    in_=xp[:, ds(r0 * W, nr * W)],
                )
                for dst, src in side.get(ci, []):
                    nc.sync.dma_start(out=dst, in_=src)
            nc.sync.dma_start(out=wct_sb, in_=wct.rearrange("i (k o) -> i k o", k=9))
            # pcomb/whw ride the gpsimd ring: ~9KB total, needed only at
            # ~+14us, and keeping the sync ring at 9 triggers stays within
            # the DMA semaphore pool (no reuse-wait stalls on x triggers).
            nc.gpsimd.dma_start(out=pc_sb, in_=pcomb[:, :])
            nc.gpsimd.dma_start(out=whw_sb, in_=whw[:, :])
            wht_sb = whw_sb[:, 0:C]
            wwt_sb = whw_sb[:, C : 2 * C]
            p128_sb = pc_sb[:, 0:5]
            p8_sb = pc_sb[0:MIP, 6:10]

            # warm fodder first so the PE can start ramping ASAP
            nc.vector.memset(wtile, 0.0)
            # conv padding border of ug
            nc.vector.memset(ug[:, 0, :], 0.0)
            nc.vector.memset(ug[:, HP - 1, :], 0.0)
            nc.vector.memset(ug[:, 1 : HP - 1, 0:2], 0.0)
            nc.vector.memset(ug[:, 1 : HP - 1, WP - 2 : WP], 0.0)

            # ACT tables: preload the sigmoid set for the a_w / a_h(0-63)
            # activations (critical path); the compiler inserts ONE switch to
            # the silu set after the last sigmoid, landing in the ACT-idle
            # window before the first conv Silu. The mid-conv a_h(64-127)
            # activation uses Tanh, which the silu set also contains.
            dump = small.tile([C, 2], F32)
            nc.scalar.activation(dump, wtile[:, 0:2], AF.Sigmoid, bias=0.0, scale=1.0)

            with tc.tile_pool(name="psA", bufs=1, space="PSUM") as psA:
                ps_yh = psA.tile([MIP, 64], F32, tag="yh")
                ps_ah = psA.tile([C, H], F32, tag="ah")
                ps_aw = psA.tile([C, W], F32, tag="aw")
                ah_sb = small.tile([C, H], BF16)
                ah_t = small.tile([C, H], F32)  # raw tanh before affine

                def emit_tree(r0, nr):
                    # 32-col segment sums for rows [r0, r0+nr) (rows >= 64).
                    # DVE only: Pool elementwise ops halve concurrent PE/DVE
                    # throughput (SBUF contention), so Pool stays idle.
                    eng = nc.vector
                    xc = xs[:, ds(r0 * W, nr * W)].rearrange(
                        "p (y q s) -> p y q s", q=4, s=32
                    )
                    t1 = pp.tile([C, 16, 4, 16], BF16, tag="t1")
                    eng.tensor_add(t1[:, :nr], xc[:, :, :, 0:16], xc[:, :, :, 16:32])
                    t2 = pp.tile([C, 16, 4, 8], BF16, tag="t2")
                    eng.tensor_add(t2[:, :nr], t1[:, :nr, :, 0:8], t1[:, :nr, :, 8:16])
                    t3 = pp.tile([C, 16, 4, 4], BF16, tag="t3")
                    eng.tensor_add(t3[:, :nr], t2[:, :nr, :, 0:4], t2[:, :nr, :, 4:8])
                    t4 = pp.tile([C, 16, 4, 2], BF16, tag="t4")
                    eng.tensor_add(t4[:, :nr], t3[:, :nr, :, 0:2], t3[:, :nr, :, 2:4])
                    eng.tensor_add(
                        s32b[:, ds(r0 - 64, nr), :], t4[:, :nr, :, 0], t4[:, :nr, :, 1]
                    )

                def bn_hswish(src, dst, n, eng=None):
                    # dst = h_swish(s1*src + t1f) / 6 in THREE ops (the /6 is
                    # repaid by scaling the downstream wh/ww weights by 6):
                    # v = (u+3)/6, w = clamp(v,0,1), dst = (v-0.5)*w
                    eng = eng or nc.vector
                    v = pp.tile([MIP, n], F32, tag="bn_v")
                    eng.tensor_scalar(
                        out=v, in0=src, scalar1=p8_sb[:, 0:1],
                        scalar2=p8_sb[:, 1:2], op0=ALU.mult, op1=ALU.add,
                    )
                    w = pp.tile([MIP, n], F32, tag="bn_w")
                    eng.tensor_scalar(
                        out=w, in0=v, scalar1=0.0, scalar2=1.0,
                        op0=ALU.max, op1=ALU.min,
                    )
                    eng.scalar_tensor_tensor(
                        out=dst, in0=v, scalar=0.5, in1=w,
                        op0=ALU.subtract, op1=ALU.mult,
                    )

                def ah_mm_tanh(rlo, rhi, xh_sh):
                    # a_h = sigmoid(wh @ xh + bh) via 0.5+0.5*tanh(.5z+.5bh):
                    # runs mid-conv when the silu table is resident (the silu
                    # set contains Tanh, so no table switch is triggered)
                    nc.tensor.matmul(
                        ps_ah[:, ds(rlo, rhi - rlo)], wht_sb, xh_sh,
                        start=True, stop=True,
                    )
                    nc.scalar.activation(
                        ah_t[:, ds(rlo, rhi - rlo)], ps_ah[:, ds(rlo, rhi - rlo)],
                        AF.Tanh, bias=p128_sb[:, 4:5], scale=0.5,
                    )
                    nc.vector.tensor_scalar(
                        out=ah_sb[:, ds(rlo, rhi - rlo)],
                        in0=ah_t[:, ds(rlo, rhi - rlo)],
                        scalar1=0.5, scalar2=0.5, op0=ALU.mult, op1=ALU.add,
                    )

                def gate_rows(rlo, rhi):
                    # ug rows = x * a_h[c,y] * a_w[c,x], 4 rows per op pair:
                    # pass1 multiplies by a_h (stride-0 bcast along x, 1x DVE
                    # rate) — it only needs a_h, so the vector queue's bypass
                    # window precomputes the first tg tiles while the a_w
                    # chain is still running; pass2 (packed, 2x rate) applies
                    # a_w. ~30% cheaper than per-row scalar_tensor_tensor.
                    # rows 0-3 go as two 2-row pairs: the single pass1 op
                    # that can prefire into the a_w bn chain's semaphore
                    # gaps is then ~350ns instead of ~670ns
                    if rlo == 0:
                        groups = [(0, 2), (2, 2)] + [
                            (y, 4) for y in range(4, rhi, 4)
                        ]
                    else:
                        groups = [(y, 4) for y in range(rlo, rhi, 4)]
                    for y, g in groups:
                        tg = tgp.tile([C, 4, W], BF16, tag="tg")
                        x4 = xs[:, ds(y * W, g * W)].rearrange(
                            "p (r x) -> p r x", r=g
                        )
                        ah_b = ah_sb[:, y : y + g, None].broadcast_to([C, g, W])
                        aw_b = aw_sb[:, None, :].broadcast_to([C, g, W])
                        nc.vector.tensor_mul(tg[:, :g], x4, ah_b)
                        nc.vector.tensor_mul(
                            ug[:, 1 + y : 1 + y + g, 2 : 2 + W], tg[:, :g], aw_b
                        )

                with tc.tile_pool(name="psW", bufs=1, space="PSUM") as psW:
                    ps_warm = psW.tile([C, 512], F32, tag="warm")
                    # x_w row-pool: 4-row matmuls with range-prescaled w1
                    # accumulate onto ONE (8, 4, W) psum bank
                    ps_xw = psW.tile([MIP, 4, W], F32, tag="xw")
                    # x_h col-pool for rows 0-63: per 16-row group, 8
                    # 16-col seg-matmuls (weights c1/c1/c2/c2/c3.. per seg)
                    # accumulate w1-projected partial sums
                    ps_xh0 = psW.tile([MIP, 64, 16], F32, tag="xh0")

                    def warm(n):
                        # dep-free warms: bridge PE gaps / hold the p-state
                        for _ in range(n):
                            nc.tensor.matmul(
                                ps_warm, wtile[:, 0:C], wtile,
                                start=True, stop=True,
                            )

                    def warm_tail(n):
                        # warms pinned on the LAST x chunk: fill the PE gap
                        # between the pool matmuls and the conv
                        for _ in range(n):
                            nc.tensor.matmul(
                                ps_warm, wtile[:, 0:C],
                                xs[:, ds(H * W - 512, 512)],
                                start=True, stop=True,
                            )

                    # ---- short PE ramp before chunk 0 lands
                    warm(2)

                    # ---- chunk-chasing ----
                    for r0, nr in XCH:
                        for g4 in range(r0, r0 + nr, 4):
                            nc.tensor.matmul(
                                ps_xw,
                                w1ts_sb[:, min(g4 // 32, 2), :],
                                xs[:, ds(g4 * W, 4 * W)],
                                start=(g4 == 0),
                                stop=(g4 == H - 4),
                                skip_group_check=True,
                            )
                        if r0 < 64:
                            # x_h seg-matmuls, 16-row psum slices
                            for b0 in range(r0, r0 + nr, 16):
                                nb = min(16, r0 + nr - b0)
                                xv = xs[:, ds(b0 * W, nb * W)].rearrange(
                                    "p (y q s) -> p y q s", q=8, s=16
                                )
                                for s in range(8):
                                    nc.tensor.matmul(
                                        ps_xh0[:, ds(b0, nb), :],
                                        w1ts_sb[:, min(s // 2, 2), :],
                                        xv[:, :, s, :],
                                        start=(s == 0),
                                        stop=(s == 7),
                                        skip_group_check=True,
                                    )
                        if r0 + nr == 64:
                            # rows 0-63 x_h: reduce psum -> yh directly (w1
                            # already applied), then bn + a_h matmul+sigmoid
                            yh0 = small.tile([MIP, 64], F32)
                            nc.vector.tensor_reduce(
                                out=yh0,
                                in_=ps_xh0,
                                axis=mybir.AxisListType.X,
                                op=ALU.add,
                            )
                            xhsA = pp.tile([MIP, 64], BF16, tag="xh_sh0")
                            bn_hswish(yh0, xhsA, 64)
                            nc.tensor.matmul(
                                ps_ah[:, 0:64], wht_sb, xhsA,
                                start=True, stop=True,
                            )
                            nc.scalar.activation(
                                ah_sb[:, 0:64], ps_ah[:, 0:64],
                                AF.Sigmoid, bias=p128_sb[:, 0:1], scale=1.0,
                            )

                    # ---- a_w chain: the critical path ----
                    # All-vector: trees 80-127 are deferred behind the first
                    # gates, so the vector queue is clear the moment the stop
                    # matmul lands. (Pool is NOT an option: its software ops
                    # cost ~2us for scalar-ptr / max-min, and it cannot read
                    # PSUM.)
                    xwp = small.tile([MIP, W], F32)
                    nc.vector.tensor_reduce(
                        out=xwp,
                        in_=ps_xw.rearrange("p j x -> p x j"),
                        axis=mybir.AxisListType.X,
                        op=ALU.add,
                    )
                    xw_s = small.tile([MIP, W], BF16)
                    bn_hswish(xwp, xw_s, W)
                    nc.tensor.matmul(ps_aw, wwt_sb, xw_s, start=True, stop=True)
                    aw_sb = small.tile([C, W], BF16)
                    nc.scalar.activation(
                        aw_sb, ps_aw, AF.Sigmoid, bias=p128_sb[:, 1:2], scale=1.0
                    )

                    # gates 0-63 unblock the conv ASAP; everything for rows
                    # 64-127 queues behind them (conv reaches row 64 only
                    # ~30us after it starts)
                    gate_rows(0, 64)

                    # rows 64-127 x_h path (deferred trees + combine): pooled
                    # -> yh matmul -> bn -> a_h matmul+tanh. Fully off the
                    # critical path (conv reaches row 64 ~35us later).
                    # xhpB uses integer weights 7/3/1; the stationary
                    # w1ts[2] = w1/128 then yields exactly
                    # w1 @ (7*S0 + 3*S1 + S2 + S3)/128 = w1 @ x_h-pool
                    emit_tree(64, 16)
                    emit_tree(80, 16)
                    emit_tree(96, 16)
                    emit_tree(112, 16)
                    tmpA = pp.tile([C, 64], F32, tag="tmpA")
                    nc.vector.tensor_add(tmpA, s32b[:, :, 2], s32b[:, :, 3])
                    m0 = pp.tile([C, 64], F32, tag="m0")
                    nc.vector.tensor_scalar_mul(m0, s32b[:, :, 0], 7.0)
                    m1 = pp.tile([C, 64], F32, tag="m1")
                    nc.vector.scalar_tensor_tensor(
                        out=m1, in0=s32b[:, :, 1], scalar=3.0, in1=m0,
                        op0=ALU.mult, op1=ALU.add,
                    )
                    xhpB = pp.tile([C, 64], BF16, tag="xhpB")
                    nc.vector.tensor_add(xhpB, tmpA, m1)

                    # bridge the post-pool PE idle until the conv starts;
                    # sized so the tails run dry just before the first conv
                    # matmul becomes ready (leftover tails would delay it)
                    warm_tail(14)

                # ---- 3x3 conv + BN2 + SiLU ----
                # The rows-64-127 a_h PE matmuls are spliced in after conv
                # block 10: their vector feeders complete behind gates 0-63,
                # and a stalled LDWEIGHTS earlier in the PE queue blocks all
                # later matmuls (no bypass once a stationary starts loading).
                # By block 11 the feeders are long ready, and gates 64-127
                # still land ~10us before conv block 16 consumes them.
                with (
                    tc.tile_pool(name="psB", bufs=5, space="PSUM") as psB,
                    tc.tile_pool(name="obp", bufs=4) as obp,
                ):
                    def conv_block(rb):
                        pso = psB.tile([C, 4, W], F32, tag="pso")
                        for k in range(9):
                            dy, dx = k // 3, k % 3
                            nc.tensor.matmul(
                                pso,
                                wct_sb[:, k, :],
                                ug[:, 4 * rb + dy : 4 * rb + dy + 4,
                                   1 + dx : 1 + dx + W],
                                start=(k == 0),
                                stop=(k == 8),
                            )
                        ob = obp.tile([C, 4, W], BF16, tag="ob")
                        nc.scalar.activation(
                            ob, pso, AF.Silu,
                            bias=p128_sb[:, 3:4], scale=p128_sb[:, 2:3],
                        )
                        nc.sync.dma_start(
                            out=outp[:, 4 * rb : 4 * rb + 4, :], in_=ob
                        )

                    for rb in range(12):
                        conv_block(rb)

                    nc.tensor.matmul(
                        ps_yh, w1ts_sb[:, 2, :], xhpB, start=True, stop=True
                    )
                    xhsB = pp.tile([MIP, 64], BF16, tag="xh_sh64")
                    bn_hswish(ps_yh, xhsB, 64)
                    ah_mm_tanh(64, 128, xhsB)
                    gate_rows(64, 128)

                    for rb in range(12, H // 4):
                        conv_block(rb)

    nc.compile()
    return nc


def prep_inputs(x, w1, b1, g1, be1, m1, v1, wh, bh, ww, bw, wc, bc, g2, be2, m2, v2):
    """Host-side prep: per-core input maps (weights replicated)."""
    bf = ml_dtypes.bfloat16
    N = x.shape[0]
    s1 = (g1 / np.sqrt(v1 + EPS)).astype(np.float64)
    t1f = s1 * b1 + be1 - m1 * s1
    # bn_hswish consts: v = (s1/6)*x + (t1f/6 + 0.5); cols 2-3 unused
    p8 = np.stack(
        [s1 / 6.0, t1f / 6.0 + 0.5, 0.0 * s1, 0.0 * s1], axis=1
    ).astype(np.float32)
    s2 = (g2 / np.sqrt(v2 + EPS)).astype(np.float64)
    b2 = bc * s2 + be2 - m2 * s2
    # pcomb cols: 0 bh, 1 bw (sigmoid path), 2 s2, 3 b2, 4 bh/2 (tanh
    # path); cols 6-9 rows 0-7: the p8 block
    pcomb = np.zeros((C, 10), np.float32)
    pcomb[:, 0:4] = np.stack([bh, bw, s2, b2], axis=1)
    pcomb[:, 4] = 0.5 * bh
    pcomb[0:MIP, 6:10] = p8
    cc = np.array([7.0 / 128, 3.0 / 128, 1.0 / 128])
    w1ts = np.stack([w1.T * c for c in cc], axis=1)              # (C, 3, MIP)
    shared = {
        "w1ts": np.ascontiguousarray(w1ts.reshape(C, 3 * MIP)).astype(bf),
        "whw": np.ascontiguousarray(
            6.0 * np.concatenate([wh.T, ww.T], axis=1)
        ).astype(bf),                                            # (MIP, 2C)
        "wct": np.ascontiguousarray(
            np.transpose(wc, (1, 2, 3, 0)).reshape(C, 9 * C)
        ).astype(bf),                                            # [i, (ky kx), o]
        "pcomb": pcomb,
    }
    in_maps = []
    for n in range(N):
        m = dict(shared)
        m["x"] = np.ascontiguousarray(x[n].reshape(C, H * W)).astype(bf)
        in_maps.append(m)
    return in_maps


def run(inputs, trace=False):
    if "nc" not in _CACHE:
        _CACHE["nc"] = build_nc()
    nc = _CACHE["nc"]
    in_maps = prep_inputs(**inputs)
    res = run_bass_kernel_spmd(nc, in_maps, core_ids=list(range(8)), trace=trace)
    out = np.stack([np.asarray(res.results[i]["out"]) for i in range(8)], axis=0)
    return out.astype(np.float32), res


def kernel(**inputs) -> np.ndarray:
    out, _ = run(inputs, trace=False)
    return out


# revision 52
# speedup vs baseline: 1.1902x; 1.0042x over previous
"""Trainium2 Bass kernel for CAConv2 (coordinate-attention + 3x3 conv block).

Shapes (hardcoded): x (8, 128, 128, 128) f32; data-parallel over batch,
one image per NeuronCore (8 cores).

Scheduling facts learned from HW traces / the cost model:
- PE streams conv matmuls at ~216ns/512 cols (~2.3 GHz effective) once
  warm; the 3x3 conv (32 blocks x 9 matmuls) is a ~63us floor and
  everything else must hide around it.
- Engines run their queues in order with a ~4-deep stalled-instruction
  bypass window: emission position is priority, dependencies are
  eligibility. A stalled matmul whose LDWEIGHTS has issued blocks ALL
  later matmuls, so PE matmuls with late feeders must be emitted at a
  queue position where their inputs are certainly ready (the B-path
  a_h matmuls are spliced between conv blocks 11 and 12).
- Pool (gpsimd) elementwise ops halve concurrent PE/DVE throughput
  (SBUF contention) and cannot touch PSUM; scalar-ptr / max-min Pool
  ops cost ~2us. Pool is left idle on purpose.
- DMA engines (16) sustain ~21 GB/s each (~300-340 GB/s aggregate); x
  (4MB bf16) needs ~13.5us of drain. All loads ride the sync HW ring
  (the gpsimd SW ring has ~1us wakeup and its completion semaphores
  stall behind trigger issuance); wct queues behind all of x on the
  same in-order ring. DMA triggers cost ~0.6-0.8us each on the issuing
  queue and completion semaphores fire ~1-2us after last data (engine
  stagger).
- ACT tables load per *set*: sigmoid set first for the critical a_w /
  a_h(0-63) activations; the single switch to the silu set lands in the
  ACT-idle window before the first conv Silu; the mid-conv a_h(64-127)
  sigmoid is computed as 0.5+0.5*tanh(z/2) (Tanh is in the silu set).
- Cross-engine dependency hops cost ~0.3-1us; run-to-run variance is
  ~+-1.5us (DMA ring behavior), and the device itself oscillates between
  a ~2.35GHz and a ~2.0GHz PE regime on minute timescales (conv cadence
  1965 vs 2377 ns/block) -- never judge a change on one run.
- The a_w critical chain is latency-tuned: 3-op bn+hswish (the /6 folded
  into 6x-scaled wh/ww weights), and the gating tg tiles live in a
  bufs=1 pool so at most ONE early-ready pass1 op can jump into the bn
  chain's semaphore gaps (awred->sigmoid = 2.64us, deterministic).
"""

import numpy as np
import ml_dtypes

import concourse.bacc as bacc
import concourse.tile as tile
from concourse import mybir
from concourse.bass import ds
from concourse.bass_utils import run_bass_kernel_spmd

BF16 = mybir.dt.bfloat16
F32 = mybir.dt.float32
C, H, W, MIP = 128, 128, 128, 8
WP = W + 4  # padded width: cols [2, 130) hold data, 0/1 and 130/131 are zero
HP = H + 2  # padded height: rows [1, 129) hold data
EPS = 1e-5
AF = mybir.ActivationFunctionType
ALU = mybir.AluOpType

_CACHE = {}


def build_nc():
    nc = bacc.Bacc(num_swdge_queues=1)
    xp = nc.declare_dram_parameter("x", [C, H * W], BF16, isOutput=False)
    w1ts = nc.declare_dram_parameter("w1ts", [C, 3 * MIP], BF16, isOutput=False)
    # wht | wwt packed side by side (one DMA trigger)
    whw = nc.declare_dram_parameter("whw", [MIP, 2 * C], BF16, isOutput=False)
    # wct[i, k, o] = wc[o, i, k//3, k%3]
    wct = nc.declare_dram_parameter("wct", [C, 9 * C], BF16, isOutput=False)
    # pcomb cols (all 128 rows): 0 bh, 1 bw, 2 s2, 3 b2 (= bc*s2+be2-m2*s2),
    # 4 bh/2 (tanh path); cols 6-9 (rows 0-7): s1/6, t1f/6, s1, t1f+3
    pcomb = nc.declare_dram_parameter("pcomb", [C, 10], F32, isOutput=False)
    outp = nc.declare_dram_parameter("out", [C, H, W], BF16, isOutput=True)

    with tile.TileContext(nc) as tc:
        with (
            tc.tile_pool(name="sing", bufs=1) as sing,
            tc.tile_pool(name="pp", bufs=2) as pp,
            # tg gets bufs=1 on purpose: DVE is serial so strict
            # pass1/pass2 alternation costs nothing, but it caps the
            # early-ready pass1 ops that can jump into the a_w bn chain's
            # semaphore gaps at ONE (each injection costs ~0.6us of
            # critical path)
            tc.tile_pool(name="tgp", bufs=1) as tgp,
            tc.tile_pool(name="small", bufs=1) as small,
        ):
            xs = sing.tile([C, H * W], BF16)
            ug = sing.tile([C, HP, WP], BF16)
            s32b = sing.tile([C, 64, 4], F32)  # col-segment sums rows 64-127
            wtile = sing.tile([C, 512], BF16)  # zeros; warm-matmul fodder

            # Everything rides the sync HW ring: the gpsimd SWDGE ring has a
            # ~0.9us wakeup latency and its completion semaphores stall
            # behind trigger issuance on the GpSimd sequencer. The HW ring is
            # in-order per engine, so interleaving small weight loads between
            # x chunks preserves the chunk-chase sequencing, and wct's 288KB
            # queue behind all of x (no HBM contention mid-stream).
            XCH = [(0, 16), (16, 16), (32, 32), (64, 32), (96, 16), (112, 8), (120, 8)]
            w1ts_sb = sing.tile([C, 3, MIP], BF16)
            whw_sb = sing.tile([MIP, 2 * C], BF16)
            pc_sb = sing.tile([C, 10], F32)
            wct_sb = sing.tile([C, 9, C], BF16)
            side = {
                0: [(w1ts_sb, w1ts.rearrange("c (r m) -> c r m", r=3))],
            }
            # (Tried: last x chunks on the gpsimd SW ring to dodge the tail
            # stagger — regressed ~10us; bulk transfers on the two rings do
            # not overlap cleanly. Tiny side loads are fine there though.)
            for ci, (r0, nr) in enumerate(XCH):
                nc.sync.dma_start(
                    out=xs[:, ds(r0 * W, nr * W)],
                    in_=xp[:, ds(r0 * W, nr * W)],
                )
                for dst, src in side.get(ci, []):
                    nc.sync.dma_start(out=dst, in_=src)
            nc.sync.dma_start(out=wct_sb, in_=wct.rearrange("i (k o) -> i k o", k=9))
            # pcomb/whw ride the gpsimd ring: ~9KB total, needed only at
            # ~+14us, and keeping the sync ring at 9 triggers stays within
            # the DMA semaphore pool (no reuse-wait stalls on x triggers).
            nc.gpsimd.dma_start(out=pc_sb, in_=pcomb[:, :])
            nc.gpsimd.dma_start(out=whw_sb, in_=whw[:, :])
            wht_sb = whw_sb[:, 0:C]
            wwt_sb = whw_sb[:, C : 2 * C]
            p128_sb = pc_sb[:, 0:5]
            p8_sb = pc_sb[0:MIP, 6:10]

            # warm fodder first so the PE can start ramping ASAP
            nc.vector.memset(wtile, 0.0)
            # conv padding border of ug
            nc.vector.memset(ug[:, 0, :], 0.0)
            nc.vector.memset(ug[:, HP - 1, :], 0.0)
            nc.vector.memset(ug[:, 1 : HP - 1, 0:2], 0.0)
            nc.vector.memset(ug[:, 1 : HP - 1, WP - 2 : WP], 0.0)

            # ACT tables: preload the sigmoid set for the a_w / a_h(0-63)
            # activations (critical path); the compiler inserts ONE switch to
            # the silu set after the last sigmoid, landing in the ACT-idle
            # window before the first conv Silu. The mid-conv a_h(64-127)
            # activation uses Tanh, which the silu set also contains.
            dump = small.tile([C, 2], F32)
            nc.scalar.activation(dump, wtile[:, 0:2], AF.Sigmoid, bias=0.0, scale=1.0)

            with tc.tile_pool(name="psA", bufs=1, space="PSUM") as psA:
                ps_yh = psA.tile([MIP, 64], F32, tag="yh")
                ps_ah = psA.tile([C, H], F32, tag="ah")
                ps_aw = psA.tile([C, W], F32, tag="aw")
                ah_sb = small.tile([C, H], BF16)
                ah_t = small.tile([C, H], F32)  # raw tanh before affine

                def emit_tree(r0, nr):
                    # 32-col segment sums for rows [r0, r0+nr) (rows >= 64).
                    # DVE only: Pool elementwise ops halve concurrent PE/DVE
                    # throughput (SBUF contention), so Pool stays idle.
                    eng = nc.vector
                    xc = xs[:, ds(r0 * W, nr * W)].rearrange(
                        "p (y q s) -> p y q s", q=4, s=32
                    )
                    t1 = pp.tile([C, 16, 4, 16], BF16, tag="t1")
                    eng.tensor_add(t1[:, :nr], xc[:, :, :, 0:16], xc[:, :, :, 16:32])
                    t2 = pp.tile([C, 16, 4, 8], BF16, tag="t2")
                    eng.tensor_add(t2[:, :nr], t1[:, :nr, :, 0:8], t1[:, :nr, :, 8:16])
                    t3 = pp.tile([C, 16, 4, 4], BF16, tag="t3")
                    eng.tensor_add(t3[:, :nr], t2[:, :nr, :, 0:4], t2[:, :nr, :, 4:8])
                    t4 = pp.tile([C, 16, 4, 2], BF16, tag="t4")
                    eng.tensor_add(t4[:, :nr], t3[:, :nr, :, 0:2], t3[:, :nr, :, 2:4])
                    eng.tensor_add(
                        s32b[:, ds(r0 - 64, nr), :], t4[:, :nr, :, 0], t4[:, :nr, :, 1]
                    )

                def bn_hswish(src, dst, n, eng=None):
                    # dst = h_swish(s1*src + t1f) / 6 in THREE ops (the /6 is
                    # repaid by scaling the downstream wh/ww weights by 6):
                    # v = (u+3)/6, w = clamp(v,0,1), dst = (v-0.5)*w
                    eng = eng or nc.vector
                    v = pp.tile([MIP, n], F32, tag="bn_v")
                    eng.tensor_scalar(
                        out=v, in0=src, scalar1=p8_sb[:, 0:1],
                        scalar2=p8_sb[:, 1:2], op0=ALU.mult, op1=ALU.add,
                    )
                    w = pp.tile([MIP, n], F32, tag="bn_w")
                    eng.tensor_scalar(
                        out=w, in0=v, scalar1=0.0, scalar2=1.0,
                        op0=ALU.max, op1=ALU.min,
                    )
                    eng.scalar_tensor_tensor(
                        out=dst, in0=v, scalar=0.5, in1=w,
                        op0=ALU.subtract, op1=ALU.mult,
                    )

                def ah_mm_tanh(rlo, rhi, xh_sh):
                    # a_h = sigmoid(wh @ xh + bh) via 0.5+0.5*tanh(.5z+.5bh):
                    # runs mid-conv when the silu table is resident (the silu
                    # set contains Tanh, so no table switch is triggered)
                    nc.tensor.matmul(
                        ps_ah[:, ds(rlo, rhi - rlo)], wht_sb, xh_sh,
                        start=True, stop=True,
                    )
                    nc.scalar.activation(
                        ah_t[:, ds(rlo, rhi - rlo)], ps_ah[:, ds(rlo, rhi - rlo)],
                        AF.Tanh, bias=p128_sb[:, 4:5], scale=0.5,
                    )
                    nc.vector.tensor_scalar(
                        out=ah_sb[:, ds(rlo, rhi - rlo)],
                        in0=ah_t[:, ds(rlo, rhi - rlo)],
                        scalar1=0.5, scalar2=0.5, op0=ALU.mult, op1=ALU.add,
                    )

                def gate_rows(rlo, rhi):
                    # ug rows = x * a_h[c,y] * a_w[c,x], 4 rows per op pair:
                    # pass1 multiplies by a_h (stride-0 bcast along x, 1x DVE
                    # rate) — it only needs a_h, so the vector queue's bypass
                    # window precomputes the first tg tiles while the a_w
                    # chain is still running; pass2 (packed, 2x rate) applies
                    # a_w. ~30% cheaper than per-row scalar_tensor_tensor.
                    # (Tried 2-row leading pairs: the bn-chain injection
                    # shrinks ~0.2us but the bufs=1 serialization delays the
                    # rows-0-3 completion ~0.8us. 4-row groups win.)
                    aw_b = aw_sb[:, None, :].broadcast_to([C, 4, W])
                    for y in range(rlo, rhi, 4):
                        tg = tgp.tile([C, 4, W], BF16, tag="tg")
                        x4 = xs[:, ds(y * W, 4 * W)].rearrange(
                            "p (r x) -> p r x", r=4
                        )
                        ah_b = ah_sb[:, y : y + 4, None].broadcast_to([C, 4, W])
                        nc.vector.tensor_mul(tg, x4, ah_b)
                        nc.vector.tensor_mul(
                            ug[:, 1 + y : 5 + y, 2 : 2 + W], tg, aw_b
                        )

                with tc.tile_pool(name="psW", bufs=1, space="PSUM") as psW:
                    ps_warm = psW.tile([C, 512], F32, tag="warm")
                    # x_w row-pool: 4-row matmuls with range-prescaled w1
                    # accumulate onto ONE (8, 4, W) psum bank
                    ps_xw = psW.tile([MIP, 4, W], F32, tag="xw")
                    # x_h col-pool for rows 0-63: per 16-row group, 8
                    # 16-col seg-matmuls (weights c1/c1/c2/c2/c3.. per seg)
                    # accumulate w1-projected partial sums
                    ps_xh0 = psW.tile([MIP, 64, 16], F32, tag="xh0")

                    def warm(n):
                        # dep-free warms: bridge PE gaps / hold the p-state
                        for _ in range(n):
                            nc.tensor.matmul(
                                ps_warm, wtile[:, 0:C], wtile,
                                start=True, stop=True,
                            )

                    def warm_tail(n):
                        # warms pinned on the LAST x chunk: fill the PE gap
                        # between the pool matmuls and the conv
                        for _ in range(n):
                            nc.tensor.matmul(
                                ps_warm, wtile[:, 0:C],
                                xs[:, ds(H * W - 512, 512)],
                                start=True, stop=True,
                            )

                    # ---- short PE ramp before chunk 0 lands
                    warm(2)

                    # ---- chunk-chasing ----
                    for r0, nr in XCH:
                        for g4 in range(r0, r0 + nr, 4):
                            nc.tensor.matmul(
                                ps_xw,
                                w1ts_sb[:, min(g4 // 32, 2), :],
                                xs[:, ds(g4 * W, 4 * W)],
                                start=(g4 == 0),
                                stop=(g4 == H - 4),
                                skip_group_check=True,
                            )
                        if r0 < 64:
                            # x_h seg-matmuls, 16-row psum slices
                            for b0 in range(r0, r0 + nr, 16):
                                nb = min(16, r0 + nr - b0)
                                xv = xs[:, ds(b0 * W, nb * W)].rearrange(
                                    "p (y q s) -> p y q s", q=8, s=16
                                )
                                for s in range(8):
                                    nc.tensor.matmul(
                                        ps_xh0[:, ds(b0, nb), :],
                                        w1ts_sb[:, min(s // 2, 2), :],
                                        xv[:, :, s, :],
                                        start=(s == 0),
                                        stop=(s == 7),
                                        skip_group_check=True,
                                    )
                        if r0 + nr == 64:
                            # rows 0-63 x_h: reduce psum -> yh directly (w1
                            # already applied), then bn + a_h matmul+sigmoid
                            yh0 = small.tile([MIP, 64], F32)
                            nc.vector.tensor_reduce(
                                out=yh0,
                                in_=ps_xh0,
                                axis=mybir.AxisListType.X,
                                op=ALU.add,
                            )
                            xhsA = pp.tile([MIP, 64], BF16, tag="xh_sh0")
                            bn_hswish(yh0, xhsA, 64)
                            nc.tensor.matmul(
                                ps_ah[:, 0:64], wht_sb, xhsA,
                                start=True, stop=True,
                            )
                            nc.scalar.activation(
                                ah_sb[:, 0:64], ps_ah[:, 0:64],
                                AF.Sigmoid, bias=p128_sb[:, 0:1], scale=1.0,
                            )

                    # ---- a_w chain: the critical path ----
                    # All-vector: trees 80-127 are deferred behind the first
                    # gates, so the vector queue is clear the moment the stop
                    # matmul lands. (Pool is NOT an option: its software ops
                    # cost ~2us for scalar-ptr / max-min, and it cannot read
                    # PSUM.)
                    xwp = small.tile([MIP, W], F32)
                    nc.vector.tensor_reduce(
                        out=xwp,
                        in_=ps_xw.rearrange("p j x -> p x j"),
                        axis=mybir.AxisListType.X,
                        op=ALU.add,
                    )
                    xw_s = small.tile([MIP, W], BF16)
                    bn_hswish(xwp, xw_s, W)
                    nc.tensor.matmul(ps_aw, wwt_sb, xw_s, start=True, stop=True)
                    aw_sb = small.tile([C, W], BF16)
                    nc.scalar.activation(
                        aw_sb, ps_aw, AF.Sigmoid, bias=p128_sb[:, 1:2], scale=1.0
                    )

                    # gates 0-63 unblock the conv ASAP; everything for rows
                    # 64-127 queues behind them (conv reaches row 64 only
                    # ~30us after it starts)
                    gate_rows(0, 64)

                    # rows 64-127 x_h path (deferred trees + combine): pooled
                    # -> yh matmul -> bn -> a_h matmul+tanh. Fully off the
                    # critical path (conv reaches row 64 ~35us later).
                    # xhpB uses integer weights 7/3/1; the stationary
                    # w1ts[2] = w1/128 then yields exactly
                    # w1 @ (7*S0 + 3*S1 + S2 + S3)/128 = w1 @ x_h-pool
                    emit_tree(64, 16)
                    emit_tree(80, 16)
                    emit_tree(96, 16)
                    emit_tree(112, 16)
                    tmpA = pp.tile([C, 64], F32, tag="tmpA")
                    nc.vector.tensor_add(tmpA, s32b[:, :, 2], s32b[:, :, 3])
                    m0 = pp.tile([C, 64], F32, tag="m0")
                    nc.vector.tensor_scalar_mul(m0, s32b[:, :, 0], 7.0)
                    m1 = pp.tile([C, 64], F32, tag="m1")
                    nc.vector.scalar_tensor_tensor(
                        out=m1, in0=s32b[:, :, 1], scalar=3.0, in1=m0,
                        op0=ALU.mult, op1=ALU.add,
                    )
                    xhpB = pp.tile([C, 64], BF16, tag="xhpB")
                    nc.vector.tensor_add(xhpB, tmpA, m1)

                    # bridge the post-pool PE idle until the conv starts;
                    # sized so the tails run dry just before the first conv
                    # matmul becomes ready (leftover tails would delay it)
                    warm_tail(14)

                # ---- 3x3 conv + BN2 + SiLU ----
                # The rows-64-127 a_h PE matmuls are spliced in after conv
                # block 10: their vector feeders complete behind gates 0-63,
                # and a stalled LDWEIGHTS earlier in the PE queue blocks all
                # later matmuls (no bypass once a stationary starts loading).
                # By block 11 the feeders are long ready, and gates 64-127
                # still land ~10us before conv block 16 consumes them.
                with (
                    tc.tile_pool(name="psB", bufs=5, space="PSUM") as psB,
                    tc.tile_pool(name="obp", bufs=4) as obp,
                ):
                    def conv_block(rb):
                        pso = psB.tile([C, 4, W], F32, tag="pso")
                        for k in range(9):
                            dy, dx = k // 3, k % 3
                            nc.tensor.matmul(
                                pso,
                                wct_sb[:, k, :],
                                ug[:, 4 * rb + dy : 4 * rb + dy + 4,
                                   1 + dx : 1 + dx + W],
                                start=(k == 0),
                                stop=(k == 8),
                            )
                        ob = obp.tile([C, 4, W], BF16, tag="ob")
                        nc.scalar.activation(
                            ob, pso, AF.Silu,
                            bias=p128_sb[:, 3:4], scale=p128_sb[:, 2:3],
                        )
                        nc.sync.dma_start(
                            out=outp[:, 4 * rb : 4 * rb + 4, :], in_=ob
                        )

                    for rb in range(12):
                        conv_block(rb)

                    nc.tensor.matmul(
                        ps_yh, w1ts_sb[:, 2, :], xhpB, start=True, stop=True
                    )
                    xhsB = pp.tile([MIP, 64], BF16, tag="xh_sh64")
                    bn_hswish(ps_yh, xhsB, 64)
                    ah_mm_tanh(64, 128, xhsB)
                    gate_rows(64, 128)

                    for rb in range(12, H // 4):
                        conv_block(rb)

    nc.compile()
    return nc


def prep_inputs(x, w1, b1, g1, be1, m1, v1, wh, bh, ww, bw, wc, bc, g2, be2, m2, v2):
    """Host-side prep: per-core input maps (weights replicated)."""
    bf = ml_dtypes.bfloat16
    N = x.shape[0]
    s1 = (g1 / np.sqrt(v1 + EPS)).astype(np.float64)
    t1f = s1 * b1 + be1 - m1 * s1
    # bn_hswish consts: v = (s1/6)*x + (t1f/6 + 0.5); cols 2-3 unused
    p8 = np.stack(
        [s1 / 6.0, t1f / 6.0 + 0.5, 0.0 * s1, 0.0 * s1], axis=1
    ).astype(np.float32)
    s2 = (g2 / np.sqrt(v2 + EPS)).astype(np.float64)
    b2 = bc * s2 + be2 - m2 * s2
    # pcomb cols: 0 bh, 1 bw (sigmoid path), 2 s2, 3 b2, 4 bh/2 (tanh
    # path); cols 6-9 rows 0-7: the p8 block
    pcomb = np.zeros((C, 10), np.float32)
    pcomb[:, 0:4] = np.stack([bh, bw, s2, b2], axis=1)
    pcomb[:, 4] = 0.5 * bh
    pcomb[0:MIP, 6:10] = p8
    cc = np.array([7.0 / 128, 3.0 / 128, 1.0 / 128])
    w1ts = np.stack([w1.T * c for c in cc], axis=1)              # (C, 3, MIP)
    shared = {
        "w1ts": np.ascontiguousarray(w1ts.reshape(C, 3 * MIP)).astype(bf),
        "whw": np.ascontiguousarray(
            6.0 * np.concatenate([wh.T, ww.T], axis=1)
        ).astype(bf),                                            # (MIP, 2C)
        "wct": np.ascontiguousarray(
            np.transpose(wc, (1, 2, 3, 0)).reshape(C, 9 * C)
        ).astype(bf),                                            # [i, (ky kx), o]
        "pcomb": pcomb,
    }
    in_maps = []
    for n in range(N):
        m = dict(shared)
        m["x"] = np.ascontiguousarray(x[n].reshape(C, H * W)).astype(bf)
        in_maps.append(m)
    return in_maps


def run(inputs, trace=False):
    if "nc" not in _CACHE:
        _CACHE["nc"] = build_nc()
    nc = _CACHE["nc"]
    in_maps = prep_inputs(**inputs)
    res = run_bass_kernel_spmd(nc, in_maps, core_ids=list(range(8)), trace=trace)
    out = np.stack([np.asarray(res.results[i]["out"]) for i in range(8)], axis=0)
    return out.astype(np.float32), res


def kernel(**inputs) -> np.ndarray:
    out, _ = run(inputs, trace=False)
    return out
